# revision 11
# baseline (speedup 1.0000x reference)
"""GTN-Rec fused kernel for 8 Trainium2 NeuronCores.

Math (only channel 0 of H is consumed downstream):
    a0 = A @ softmax(W1a)[0]; b0 = A @ softmax(W1b)[0]; a2 = A @ softmax(W2)[0]
    y3 = ((x @ a0) @ b0) @ a2          (x = seqs flattened [B*S, N])
    enc = x*relu(I_B) + relu(y3 - thr)
    basket = relu(enc @ lin_w.T + lin_b) -> LSTM -> take h[seq_len-1]
    predict = sigmoid(last @ Wscore.T) * (0.5 + 0.5*relu(I_B))

Sharding: A/a0/b0/a2 column-sharded (N/8=250 per core) for stage 0+1;
one AllToAll converts the stage-1 output from column-sharded to
batch-sharded (240 of B*S rows per core); b0/a2 are AllGathered early
(off critical path) so stages 2-3 run with full contraction and no
further activation collectives; LSTM + scoring run batch-sharded
(8 of 64 batches per core); host concatenates the row shards.

All matmuls in bf16 (measured end-to-end rel err ~4e-4 vs fp32 ref).
"""
import numpy as np
import ml_dtypes

N, E, C, D, U, B, S = 2000, 3, 2, 128, 128, 64, 30
BT = B * S            # 1920
NCORES = 8
NSH = N // NCORES     # 250
BSH = B // NCORES     # 8
BTS = BSH * S         # 240
NT = (N + 127) // 128  # 16 K/M tiles over N (15x128 + 80)
CH = 4                # moving chunks for 1920-wide stage-1 output
CW = BT // CH         # 480

_bf16 = ml_dtypes.bfloat16


def _ntile(i):
    """(row0, width) of the i-th 128-row tile over N=2000."""
    r0 = i * 128
    return r0, min(128, N - r0)


def _build_program(debug_taps=False):
    import concourse.mybir as mybir
    import concourse.tile as tile
    from concourse import bacc

    dt = mybir.dt
    nc = bacc.Bacc("TRN2", target_bir_lowering=False, debug=False,
                   num_devices=NCORES)

    # ---- I/O ----
    planes_d = nc.dram_tensor("planes", [N, E, NSH], dt.bfloat16, kind="ExternalInput")
    xT_d = nc.dram_tensor("xT", [N, BT], dt.bfloat16, kind="ExternalInput")
    xsT_d = nc.dram_tensor("xsT", [N, BTS], dt.bfloat16, kind="ExternalInput")
    wvec_d = nc.dram_tensor("wvec", [128, 3 * E], dt.float32, kind="ExternalInput")
    thr_d = nc.dram_tensor("thr", [128, 1], dt.float32, kind="ExternalInput")
    linwT_d = nc.dram_tensor("linwT", [N, D], dt.bfloat16, kind="ExternalInput")
    linb_d = nc.dram_tensor("linb", [128, 1], dt.float32, kind="ExternalInput")
    wih_d = nc.dram_tensor("wih", [128, 4 * U], dt.bfloat16, kind="ExternalInput")
    whh_d = nc.dram_tensor("whh", [128, 4 * U], dt.bfloat16, kind="ExternalInput")
    biasp_d = nc.dram_tensor("biasp", [128, 4], dt.float32, kind="ExternalInput")
    wsc_d = nc.dram_tensor("wsc", [128, N], dt.bfloat16, kind="ExternalInput")
    factor_d = nc.dram_tensor("factor", [BSH, N], dt.float32, kind="ExternalInput")
    h0t_d = nc.dram_tensor("h0t", [128, BSH], dt.float32, kind="ExternalInput")
    c0t_d = nc.dram_tensor("c0t", [128, BSH], dt.float32, kind="ExternalInput")
    mask_d = nc.dram_tensor("maskt", [128, S, BSH], dt.float32, kind="ExternalInput")
    out_d = nc.dram_tensor("predict", [BSH, N], dt.float32, kind="ExternalOutput")
    if debug_taps:
        dbg_a0 = nc.dram_tensor("dbg_a0", [N, NSH], dt.bfloat16, kind="ExternalOutput")
        dbg_y1 = nc.dram_tensor("dbg_y1", [128, 2, BT], dt.bfloat16, kind="ExternalOutput")
        dbg_y1o = nc.dram_tensor("dbg_y1o", [N, BTS], dt.bfloat16, kind="ExternalOutput")
        dbg_y2 = nc.dram_tensor("dbg_y2", [128, NT, BTS], dt.bfloat16, kind="ExternalOutput")
        dbg_enc = nc.dram_tensor("dbg_enc", [128, NT, BTS], dt.bfloat16, kind="ExternalOutput")
        dbg_bk = nc.dram_tensor("dbg_bk", [128, BTS], dt.bfloat16, kind="ExternalOutput")
        dbg_gx = nc.dram_tensor("dbg_gx", [128, 4, BTS], dt.float32, kind="ExternalOutput")
        dbg_last = nc.dram_tensor("dbg_last", [128, BSH], dt.float32, kind="ExternalOutput")

    # ---- internal DRAM (collective bounce) ----
    b0cc_in = nc.dram_tensor("b0cc_in", [N, NSH], dt.bfloat16)
    a2cc_in = nc.dram_tensor("a2cc_in", [N, NSH], dt.bfloat16)
    b0full = nc.dram_tensor("b0full", [NCORES, N, NSH], dt.bfloat16,
                            addr_space="Shared")
    a2full = nc.dram_tensor("a2full", [NCORES, N, NSH], dt.bfloat16,
                            addr_space="Shared")
    y1cc_in = nc.dram_tensor("y1cc_in", [N, BTS], dt.bfloat16)
    y1cc_out = nc.dram_tensor("y1cc_out", [N, BTS], dt.bfloat16)

    rg = [list(range(NCORES))]

    with tile.TileContext(nc) as tc:
        with tc.tile_pool(name="sb", bufs=3) as sb, \
             tc.tile_pool(name="res", bufs=1) as res, \
             tc.tile_pool(name="ps", bufs=8, space="PSUM") as ps:

            # ---- constants in SBUF ----
            wvec_t = res.tile([128, 3 * E], dt.float32, tag="wvec")
            nc.sync.dma_start(wvec_t[:], wvec_d[:])
            thr_t = res.tile([128, 1], dt.float32, tag="thr")
            nc.sync.dma_start(thr_t[:], thr_d[:])
            linb_t = res.tile([128, 1], dt.float32, tag="linb")
            nc.sync.dma_start(linb_t[:], linb_d[:])
            wih_t = res.tile([128, 4 * U], dt.bfloat16, tag="wih")
            nc.sync.dma_start(wih_t[:], wih_d[:])
            whh_t = res.tile([128, 4 * U], dt.bfloat16, tag="whh")
            nc.sync.dma_start(whh_t[:], whh_d[:])
            biasp_t = res.tile([128, 4], dt.float32, tag="biasp")
            nc.sync.dma_start(biasp_t[:], biasp_d[:])
            mask_t = res.tile([128, S, BSH], dt.float32, tag="mask")
            nc.sync.dma_start(mask_t[:], mask_d[:])
            factor_t = res.tile([BSH, N], dt.float32, tag="factor")
            nc.sync.dma_start(factor_t[:], factor_d[:])

            # ---- stage 0: a0/b0/a2 column shards from A planes ----
            # + stage 1: y1T_k[250, 1920] = a0_k^T @ x^T   (K = n, 16 tiles)
            y1sb = res.tile([128, 2, BT], dt.bfloat16, tag="y1sb")
            ps1 = [ps.tile([128, 512], dt.float32, tag="psbig", name=f"ps1_{q}") for q in range(8)]
            for i in range(NT):
                r0, kw = _ntile(i)
                pl = sb.tile([128, E, NSH], dt.bfloat16, tag="planes")
                nc.sync.dma_start(pl[:kw], planes_d[r0:r0 + kw])
                xt = sb.tile([128, BT], dt.bfloat16, tag="xT")
                nc.sync.dma_start(xt[:kw], xT_d[r0:r0 + kw])

                a0t = sb.tile([128, NSH], dt.bfloat16, tag="a0")
                b0t = sb.tile([128, NSH], dt.bfloat16, tag="b0")
                a2t = sb.tile([128, NSH], dt.bfloat16, tag="a2")
                for j, ot in enumerate((a0t, b0t, a2t)):
                    t_a = sb.tile([128, NSH], dt.float32, tag="tmp0a")
                    t_b = sb.tile([128, NSH], dt.float32, tag="tmp0b")
                    nc.vector.tensor_scalar(
                        t_a[:kw], pl[:kw, 0, :], wvec_t[:kw, 3 * j:3 * j + 1],
                        None, mybir.AluOpType.mult)
                    nc.vector.scalar_tensor_tensor(
                        t_b[:kw], pl[:kw, 1, :], wvec_t[:kw, 3 * j + 1:3 * j + 2],
                        t_a[:kw], mybir.AluOpType.mult, mybir.AluOpType.add)
                    nc.vector.scalar_tensor_tensor(
                        ot[:kw], pl[:kw, 2, :], wvec_t[:kw, 3 * j + 2:3 * j + 3],
                        t_b[:kw], mybir.AluOpType.mult, mybir.AluOpType.add)
                if debug_taps:
                    nc.sync.dma_start(dbg_a0[r0:r0 + kw], a0t[:kw])
                # ship b0/a2 shards out for the early AllGathers
                nc.sync.dma_start(b0cc_in[r0:r0 + kw], b0t[:kw])
                nc.sync.dma_start(a2cc_in[r0:r0 + kw], a2t[:kw])
                # stage-1 matmuls, accumulate over i
                for m2 in range(2):
                    mw = 128 if m2 == 0 else NSH - 128
                    for c in range(CH):
                        nc.tensor.matmul(
                            ps1[m2 * CH + c][:mw, :CW],
                            a0t[:kw, m2 * 128:m2 * 128 + mw],
                            xt[:kw, c * CW:(c + 1) * CW],
                            start=(i == 0), stop=(i == NT - 1))

            nc.gpsimd.collective_compute(
                "AllGather", mybir.AluOpType.bypass, replica_groups=rg,
                ins=[b0cc_in[:]], outs=[b0full[:]])
            nc.gpsimd.collective_compute(
                "AllGather", mybir.AluOpType.bypass, replica_groups=rg,
                ins=[a2cc_in[:]], outs=[a2full[:]])

            # evacuate stage-1 psum -> bf16, ship to AllToAll bounce
            for m2 in range(2):
                mw = 128 if m2 == 0 else NSH - 128
                for c in range(CH):
                    nc.vector.tensor_copy(
                        y1sb[:mw, m2, c * CW:(c + 1) * CW],
                        ps1[m2 * CH + c][:mw, :CW])
            for c in range(NCORES):
                for m2 in range(2):
                    mw = 128 if m2 == 0 else NSH - 128
                    nc.sync.dma_start(
                        y1cc_in[c * NSH + m2 * 128:c * NSH + m2 * 128 + mw],
                        y1sb[:mw, m2, c * BTS:(c + 1) * BTS])
            nc.gpsimd.collective_compute(
                "AllToAll", mybir.AluOpType.bypass, replica_groups=rg,
                ins=[y1cc_in[:]], outs=[y1cc_out[:]])
            if debug_taps:
                nc.sync.dma_start(dbg_y1[:], y1sb[:])

            # ---- stage 2: y2T_own[2000, 240] = b0^T @ y1T_own ----
            # one accumulation group per PSUM bank (start=True clears the
            # whole bank's has_written bits) -> two passes of 8 M-tiles
            y2sb = res.tile([128, NT, BTS], dt.bfloat16, tag="y2sb")
            HALves = [(0, 0, 5), (1, 1000, 4)]
            for hp, mbase, nb in HALves:
                psh = [ps.tile([128, 512], dt.float32, tag="psbig",
                               name=f"ps2_{hp}_{q}") for q in range(8)]
                c0 = mbase // NSH
                for i in range(NT):
                    r0, kw = _ntile(i)
                    brow = sb.tile([128, 5 * NSH], dt.bfloat16, tag="brow")
                    for cb in range(nb):
                        nc.sync.dma_start(
                            brow[:kw, cb * NSH:(cb + 1) * NSH],
                            b0full[c0 + cb, r0:r0 + kw, :])
                    y1t = sb.tile([128, BTS], dt.bfloat16, tag="y1t")
                    nc.sync.dma_start(y1t[:kw], y1cc_out[r0:r0 + kw])
                    for m8 in range(8):
                        mt = hp * 8 + m8
                        m0, mw = _ntile(mt)
                        nc.tensor.matmul(
                            psh[m8][:mw, :BTS],
                            brow[:kw, m0 - mbase:m0 - mbase + mw], y1t[:kw],
                            start=(i == 0), stop=(i == NT - 1))
                for m8 in range(8):
                    mt = hp * 8 + m8
                    m0, mw = _ntile(mt)
                    nc.vector.tensor_copy(y2sb[:mw, mt, :], psh[m8][:mw, :BTS])

            if debug_taps:
                nc.sync.dma_start(dbg_y2[:], y2sb[:])
                for i in range(NT):
                    r0, kw = _ntile(i)
                    y1dbg = sb.tile([128, BTS], dt.bfloat16, tag="y1dbg")
                    nc.sync.dma_start(y1dbg[:kw], y1cc_out[r0:r0 + kw])
                    nc.sync.dma_start(dbg_y1o[r0:r0 + kw], y1dbg[:kw])
            # ---- stage 3: y3T_own = a2^T @ y2T_own; fused enc ----
            encT = res.tile([128, NT, BTS], dt.bfloat16, tag="encT")
            for hp, mbase, nb in HALves:
                psh3 = [ps.tile([128, 512], dt.float32, tag="psbig",
                                name=f"ps3_{hp}_{q}") for q in range(8)]
                c0 = mbase // NSH
                for i in range(NT):
                    r0, kw = _ntile(i)
                    arow = sb.tile([128, 5 * NSH], dt.bfloat16, tag="arow")
                    for cb in range(nb):
                        nc.sync.dma_start(
                            arow[:kw, cb * NSH:(cb + 1) * NSH],
                            a2full[c0 + cb, r0:r0 + kw, :])
                    for m8 in range(8):
                        mt = hp * 8 + m8
                        m0, mw = _ntile(mt)
                        nc.tensor.matmul(
                            psh3[m8][:mw, :BTS],
                            arow[:kw, m0 - mbase:m0 - mbase + mw],
                            y2sb[:kw, i, :],
                            start=(i == 0), stop=(i == NT - 1))
                for m8 in range(8):
                    mt = hp * 8 + m8
                    m0, mw = _ntile(mt)
                    xst = sb.tile([128, BTS], dt.bfloat16, tag="xst")
                    nc.sync.dma_start(xst[:mw], xsT_d[m0:m0 + mw])
                    t_e = sb.tile([128, BTS], dt.float32, tag="tmpenc")
                    nc.vector.tensor_scalar(
                        t_e[:mw], psh3[m8][:mw, :BTS],
                        thr_t[:mw, 0:1], 0.0,
                        mybir.AluOpType.subtract, mybir.AluOpType.max)
                    nc.vector.tensor_tensor(
                        encT[:mw, mt, :], t_e[:mw], xst[:mw],
                        mybir.AluOpType.add)

            if debug_taps:
                nc.sync.dma_start(dbg_enc[:], encT[:])
            # ---- basket^T[128, 240] = relu(lin_w @ enc^T + lin_b) ----
            pb = ps.tile([128, BTS], dt.float32, tag="psbig", name="pb")
            for i in range(NT):
                r0, kw = _ntile(i)
                lwt = sb.tile([128, D], dt.bfloat16, tag="lwt")
                nc.sync.dma_start(lwt[:kw], linwT_d[r0:r0 + kw])
                nc.tensor.matmul(pb[:], lwt[:kw], encT[:kw, i, :],
                                 start=(i == 0), stop=(i == NT - 1))
            bkT = res.tile([128, BTS], dt.bfloat16, tag="bkT")
            nc.scalar.activation(bkT[:], pb[:],
                                 mybir.ActivationFunctionType.Relu,
                                 bias=linb_t[:, 0:1], scale=1.0)

            if debug_taps:
                nc.sync.dma_start(dbg_bk[:], bkT[:])
            # ---- gx[128, 4, 8, 30] = Wih @ basket^T + (bih+bhh) ----
            gx = res.tile([128, 4, BTS], dt.float32, tag="gx")
            for g in range(4):
                pg = ps.tile([128, BTS], dt.float32, tag="psbig", name=f"pg_{g}")
                nc.tensor.matmul(pg[:], wih_t[:, g * 128:(g + 1) * 128], bkT[:],
                                 start=True, stop=True)
                nc.vector.tensor_scalar(
                    gx[:, g], pg[:], biasp_t[:, g:g + 1], None,
                    mybir.AluOpType.add)

            if debug_taps:
                nc.sync.dma_start(dbg_gx[:], gx[:])
            # ---- LSTM over S steps, gates order [i, f, o, g] ----
            hT = res.tile([128, BSH], dt.float32, tag="hT")
            cT = res.tile([128, BSH], dt.float32, tag="cT")
            lastT = res.tile([128, BSH], dt.float32, tag="lastT")
            nc.sync.dma_start(hT[:], h0t_d[:])
            nc.sync.dma_start(cT[:], c0t_d[:])
            nc.vector.memset(lastT[:], 0.0)
            hT_bf = res.tile([128, BSH], dt.bfloat16, tag="hT_bf")
            nc.vector.tensor_copy(hT_bf[:], hT[:])
            for t in range(S):
                pgh = ps.tile([128, 4, BSH], dt.float32, tag="psbig", name=f"pgh_{t}")
                for g in range(4):
                    nc.tensor.matmul(pgh[:, g], whh_t[:, g * 128:(g + 1) * 128],
                                     hT_bf[:], start=True, stop=True)
                gall = sb.tile([128, 4, BSH], dt.float32, tag="gall")
                nc.vector.tensor_tensor(gall[:], pgh[:],
                                        gx[:, :, t * BSH:(t + 1) * BSH],
                                        mybir.AluOpType.add)
                sg = sb.tile([128, 3, BSH], dt.float32, tag="sg")
                nc.scalar.activation(sg[:], gall[:, 0:3],
                                     mybir.ActivationFunctionType.Sigmoid)
                tg = sb.tile([128, BSH], dt.float32, tag="tg")
                nc.scalar.activation(tg[:], gall[:, 3],
                                     mybir.ActivationFunctionType.Tanh)
                t1 = sb.tile([128, BSH], dt.float32, tag="t1")
                nc.vector.tensor_tensor(t1[:], sg[:, 1], cT[:],
                                        mybir.AluOpType.mult)
                t2 = sb.tile([128, BSH], dt.float32, tag="t2")
                nc.vector.tensor_tensor(t2[:], sg[:, 0], tg[:],
                                        mybir.AluOpType.mult)
                nc.vector.tensor_tensor(cT[:], t1[:], t2[:],
                                        mybir.AluOpType.add)
                tc_ = sb.tile([128, BSH], dt.float32, tag="tc_")
                nc.scalar.activation(tc_[:], cT[:],
                                     mybir.ActivationFunctionType.Tanh)
                nc.vector.tensor_tensor(hT[:], sg[:, 2], tc_[:],
                                        mybir.AluOpType.mult)
                nc.vector.tensor_copy(hT_bf[:], hT[:])
                m1 = sb.tile([128, BSH], dt.float32, tag="m1")
                nc.vector.tensor_tensor(m1[:], hT[:], mask_t[:, t],
                                        mybir.AluOpType.mult)
                nc.vector.tensor_tensor(lastT[:], lastT[:], m1[:],
                                        mybir.AluOpType.add)

            if debug_taps:
                nc.sync.dma_start(dbg_last[:], lastT[:])
            # ---- scoring: predict[8, 2000] ----
            last_bf = res.tile([128, BSH], dt.bfloat16, tag="last_bf")
            nc.vector.tensor_copy(last_bf[:], lastT[:])
            prb = res.tile([BSH, N], dt.float32, tag="prb")
            for c in range(4):
                wst = sb.tile([128, N // 4], dt.bfloat16, tag="wst")
                nc.sync.dma_start(wst[:], wsc_d[:, c * (N // 4):(c + 1) * (N // 4)])
                psc = ps.tile([BSH, N // 4], dt.float32, tag="psbig", name=f"psc_{c}")
                nc.tensor.matmul(psc[:], last_bf[:], wst[:],
                                 start=True, stop=True)
                nc.scalar.activation(prb[:, c * (N // 4):(c + 1) * (N // 4)],
                                     psc[:],
                                     mybir.ActivationFunctionType.Sigmoid)
            outsb = res.tile([BSH, N], dt.float32, tag="outsb")
            nc.vector.tensor_tensor(outsb[:], prb[:], factor_t[:],
                                    mybir.AluOpType.mult)
            nc.sync.dma_start(out_d[:], outsb[:])

    nc.compile()
    return nc


def _softmax_row0(w):
    w = np.asarray(w, np.float64)
    m = w - w.max(axis=1, keepdims=True)
    e = np.exp(m)
    p = e / e.sum(axis=1, keepdims=True)
    return p[0].astype(np.float32)


def _host_prep(inputs):
    return _host_prep_args(**inputs)


def _host_prep_args(A, seq_len, seqs, h0, c0, W1a, W1b, W2, lin_w, lin_b,
                    Wih, Whh, bih, bhh, Wscore, I_B, threshold):
    A = np.asarray(A, np.float32)
    seqs = np.asarray(seqs, np.float32)
    seq_len = np.asarray(seq_len).astype(np.int64)
    sa = _softmax_row0(W1a)
    sb_ = _softmax_row0(W1b)
    s2 = _softmax_row0(W2)
    scale = np.maximum(np.asarray(I_B, np.float32), 0.0)

    x = seqs.reshape(BT, N)
    # column order: block c (core) outer, then (t, b') t-major within block
    cperm = np.empty(BT, np.int64)
    for c_ in range(NCORES):
        for t_ in range(S):
            for b2 in range(BSH):
                cperm[c_ * BTS + t_ * BSH + b2] = (c_ * BSH + b2) * S + t_
    xTp = x.T[:, cperm]
    xT = np.ascontiguousarray(xTp).astype(_bf16)
    xsT = np.ascontiguousarray(xTp * scale[:, None]).astype(_bf16)
    wvec = np.tile(np.concatenate([sa, sb_, s2])[None, :], (128, 1)).astype(np.float32)
    thr = np.full((128, 1), np.asarray(threshold, np.float32).ravel()[0], np.float32)
    linwT = np.ascontiguousarray(np.asarray(lin_w, np.float32).T).astype(_bf16)
    linb = np.asarray(lin_b, np.float32).reshape(128, 1)

    perm = np.r_[0:128, 128:256, 384:512, 256:384]  # [i, f, o, g]
    wih = np.ascontiguousarray(np.asarray(Wih, np.float32)[perm].T).astype(_bf16)
    whh = np.ascontiguousarray(np.asarray(Whh, np.float32)[perm].T).astype(_bf16)
    biasp = np.ascontiguousarray(
        (np.asarray(bih, np.float32) + np.asarray(bhh, np.float32))[perm]
        .reshape(4, 128).T).astype(np.float32)
    wsc = np.ascontiguousarray(np.asarray(Wscore, np.float32).T).astype(_bf16)
    factor = np.tile((0.5 + 0.5 * scale)[None, :], (BSH, 1)).astype(np.float32)

    in_maps = []
    for k in range(NCORES):
        ck = slice(k * NSH, (k + 1) * NSH)
        bk = slice(k * BSH, (k + 1) * BSH)
        planes = np.ascontiguousarray(
            A[:, ck, :].transpose(0, 2, 1)).astype(_bf16)
        mm = np.zeros((S, BSH), np.float32)
        sl = seq_len[bk]
        for b2 in range(BSH):
            mm[sl[b2] - 1, b2] = 1.0
        in_maps.append({
            "planes": planes,
            "xT": xT,
            "xsT": np.ascontiguousarray(xsT[:, k * BTS:(k + 1) * BTS]),
            "wvec": wvec,
            "thr": thr,
            "linwT": linwT,
            "linb": linb,
            "wih": wih,
            "whh": whh,
            "biasp": biasp,
            "wsc": wsc,
            "factor": factor,
            "h0t": np.ascontiguousarray(np.asarray(h0, np.float32)[0, bk].T),
            "c0t": np.ascontiguousarray(np.asarray(c0, np.float32)[0, bk].T),
            "maskt": np.tile(mm.reshape(1, S, BSH), (128, 1, 1)).astype(np.float32),
        })
    return in_maps


def kernel(A, seq_len, seqs, h0, c0, W1a, W1b, W2, lin_w, lin_b,
           Wih, Whh, bih, bhh, Wscore, I_B, threshold):
    from concourse.bass_utils import run_bass_kernel_spmd

    in_maps = _host_prep_args(A, seq_len, seqs, h0, c0, W1a, W1b, W2,
                              lin_w, lin_b, Wih, Whh, bih, bhh, Wscore,
                              I_B, threshold)
    nc = _build_program()
    res = run_bass_kernel_spmd(nc, in_maps, list(range(NCORES)))
    out = np.concatenate([res.results[k]["predict"] for k in range(NCORES)], axis=0)
    return out.astype(np.float32)


# revision 13
# speedup vs baseline: 206.9049x; 206.9049x over previous
"""GTN-Rec fused kernel for 8 Trainium2 NeuronCores.

Math (only channel 0 of H is consumed downstream):
    a0 = A @ softmax(W1a)[0]; b0 = A @ softmax(W1b)[0]; a2 = A @ softmax(W2)[0]
    y3 = ((x @ a0) @ b0) @ a2          (x = seqs flattened [B*S, N])
    enc = x*relu(I_B) + relu(y3 - thr)
    basket = relu(enc @ lin_w.T + lin_b) -> LSTM -> take h[seq_len-1]
    predict = sigmoid(last @ Wscore.T) * (0.5 + 0.5*relu(I_B))

Sharding: a0 column-sharded (N/8=250 cols per core) for stage 1
(y1T_k = a0_k^T x^T over all B*S); one AllToAll re-shards the stage-1
output from column- to batch-sharded (240 of B*S per core); b0/a2 are
produced ROW-sharded and AllGathered early so stages 2-3 run the full
contraction locally with no further activation collectives; LSTM +
scoring run batch-sharded (8 of 64 batches per core); the host
concatenates the row shards.

Chain matmuls in bf16 w/ fp32 PSUM (end-to-end rel err ~4e-4 vs fp32
reference); LSTM pocket in fp16.
"""
import numpy as np
import ml_dtypes

N, E, C, D, U, B, S = 2000, 3, 2, 128, 128, 64, 30
BT = B * S            # 1920
NCORES = 8
NSH = N // NCORES     # 250
BSH = B // NCORES     # 8
BTS = BSH * S         # 240
NT = (N + 127) // 128  # 16 K/M tiles over N (15x128 + 80)
CH = 4                # moving chunks for 1920-wide stage-1 output
CW = BT // CH         # 480

_bf16 = ml_dtypes.bfloat16


def _ntile(i):
    """(row0, width) of the i-th 128-row tile over N=2000."""
    r0 = i * 128
    return r0, min(128, N - r0)


def _row_segments(r0, kw):
    """Split rows [r0, r0+kw) at NSH-sized core-block boundaries."""
    segs = []
    r = r0
    while r < r0 + kw:
        hi = min(r0 + kw, (r // NSH + 1) * NSH)
        segs.append((r, hi))
        r = hi
    return segs


def _build_program(debug_taps=False):
    import concourse.mybir as mybir
    import concourse.tile as tile
    from concourse import bacc

    dt = mybir.dt
    AL = mybir.AluOpType
    AF = mybir.ActivationFunctionType
    nc = bacc.Bacc("TRN2", target_bir_lowering=False, debug=False,
                   num_devices=NCORES)

    # ---- I/O ----
    pcol_d = nc.dram_tensor("pcol", [N, E, NSH], dt.bfloat16, kind="ExternalInput")
    prow_d = nc.dram_tensor("prow", [NSH, E, N], dt.bfloat16, kind="ExternalInput")
    xT_d = nc.dram_tensor("xT", [N, BT], dt.bfloat16, kind="ExternalInput")
    xsT_d = nc.dram_tensor("xsT", [N, BTS], dt.bfloat16, kind="ExternalInput")
    wvec_d = nc.dram_tensor("wvec", [128, 3 * E], dt.float32, kind="ExternalInput")
    thr_d = nc.dram_tensor("thr", [128, 1], dt.float32, kind="ExternalInput")
    linwT_d = nc.dram_tensor("linwT", [N, D], dt.bfloat16, kind="ExternalInput")
    linb_d = nc.dram_tensor("linb", [128, 1], dt.float32, kind="ExternalInput")
    wih_d = nc.dram_tensor("wih", [128, 4 * U], dt.bfloat16, kind="ExternalInput")
    whh_d = nc.dram_tensor("whh", [128, 4 * U], dt.float16, kind="ExternalInput")
    biasp_d = nc.dram_tensor("biasp", [128, 4], dt.float32, kind="ExternalInput")
    wsc_d = nc.dram_tensor("wsc", [128, N], dt.float16, kind="ExternalInput")
    factor_d = nc.dram_tensor("factor", [BSH, N], dt.float32, kind="ExternalInput")
    h0t_d = nc.dram_tensor("h0t", [128, BSH], dt.float32, kind="ExternalInput")
    c0t_d = nc.dram_tensor("c0t", [128, BSH], dt.float32, kind="ExternalInput")
    mask_d = nc.dram_tensor("maskt", [128, S, BSH], dt.float16, kind="ExternalInput")
    out_d = nc.dram_tensor("predict", [BSH, N], dt.float32, kind="ExternalOutput")
    if debug_taps:
        dbg_y1 = nc.dram_tensor("dbg_y1", [128, 2, BT], dt.bfloat16, kind="ExternalOutput")
        dbg_y2 = nc.dram_tensor("dbg_y2", [128, NT, BTS], dt.bfloat16, kind="ExternalOutput")
        dbg_enc = nc.dram_tensor("dbg_enc", [128, NT, BTS], dt.bfloat16, kind="ExternalOutput")
        dbg_bk = nc.dram_tensor("dbg_bk", [128, BTS], dt.bfloat16, kind="ExternalOutput")
        dbg_last = nc.dram_tensor("dbg_last", [128, BSH], dt.float16, kind="ExternalOutput")

    # ---- internal DRAM (collective bounce) ----
    b0cc_in = nc.dram_tensor("b0cc_in", [NSH, N], dt.bfloat16)
    a2cc_in = nc.dram_tensor("a2cc_in", [NSH, N], dt.bfloat16)
    b0full = nc.dram_tensor("b0full", [N, N], dt.bfloat16, addr_space="Shared")
    a2full = nc.dram_tensor("a2full", [N, N], dt.bfloat16, addr_space="Shared")
    y1cc_in = nc.dram_tensor("y1cc_in", [N, BTS], dt.bfloat16)
    y1cc_out = nc.dram_tensor("y1cc_out", [N, BTS], dt.bfloat16)

    rg = [list(range(NCORES))]

    with tile.TileContext(nc) as tc:
        with tc.tile_pool(name="sb", bufs=3) as sb, \
             tc.tile_pool(name="res", bufs=1) as res, \
             tc.tile_pool(name="ps", bufs=8, space="PSUM") as ps:

            # ---- constants in SBUF ----
            wvec_t = res.tile([128, 3 * E], dt.float32, tag="wvec")
            nc.sync.dma_start(wvec_t[:], wvec_d[:])
            thr_t = res.tile([128, 1], dt.float32, tag="thr")
            nc.sync.dma_start(thr_t[:], thr_d[:])
            linb_t = res.tile([128, 1], dt.float32, tag="linb")
            nc.sync.dma_start(linb_t[:], linb_d[:])
            wih_t = res.tile([128, 4 * U], dt.bfloat16, tag="wih")
            nc.sync.dma_start(wih_t[:], wih_d[:])
            whh_t = res.tile([128, 4 * U], dt.float16, tag="whh")
            nc.sync.dma_start(whh_t[:], whh_d[:])
            biasp_t = res.tile([128, 4], dt.float32, tag="biasp")
            nc.sync.dma_start(biasp_t[:], biasp_d[:])
            mask_t = res.tile([128, S, BSH], dt.float16, tag="mask")
            nc.sync.dma_start(mask_t[:], mask_d[:])
            factor_t = res.tile([BSH, N], dt.float32, tag="factor")
            nc.sync.dma_start(factor_t[:], factor_d[:])

            # ---- stage 0 (rows): b0/a2 row shards -> early AllGathers ----
            for rt in range(2):
                p0, pw = (0, 128) if rt == 0 else (128, NSH - 128)
                plr = sb.tile([128, E, N], dt.bfloat16, tag="prow", bufs=2)
                nc.scalar.dma_start(plr[:pw], prow_d[p0:p0 + pw])
                for j, cc in ((1, b0cc_in), (2, a2cc_in)):
                    ot = sb.tile([128, N], dt.bfloat16, tag="rowout", bufs=2)
                    for half in range(2):
                        hs = slice(half * 1000, half * 1000 + 1000)
                        t_a = sb.tile([128, 1000], dt.float32, tag="tmp0a", bufs=2)
                        t_b = sb.tile([128, 1000], dt.float32, tag="tmp0b", bufs=2)
                        nc.vector.tensor_scalar(
                            t_a[:pw], plr[:pw, 0, hs],
                            wvec_t[:pw, 3 * j:3 * j + 1], None, AL.mult)
                        nc.vector.scalar_tensor_tensor(
                            t_b[:pw], plr[:pw, 1, hs],
                            wvec_t[:pw, 3 * j + 1:3 * j + 2], t_a[:pw],
                            AL.mult, AL.add)
                        nc.vector.scalar_tensor_tensor(
                            ot[:pw, hs], plr[:pw, 2, hs],
                            wvec_t[:pw, 3 * j + 2:3 * j + 3], t_b[:pw],
                            AL.mult, AL.add)
                    nc.scalar.dma_start(cc[p0:p0 + pw], ot[:pw])

            nc.gpsimd.collective_compute(
                "AllGather", AL.bypass, replica_groups=rg,
                ins=[b0cc_in[:]], outs=[b0full[:]])

            # ---- stage 0 (cols): a0 shard; stage 1: y1T_k = a0_k^T @ x^T ----
            y1sb = res.tile([128, 2, BT], dt.bfloat16, tag="y1sb")
            ps1 = [ps.tile([128, 512], dt.float32, tag="psbig", name=f"ps1_{q}")
                   for q in range(8)]
            for i in range(NT):
                r0, kw = _ntile(i)
                pl = sb.tile([128, E, NSH], dt.bfloat16, tag="pcol")
                nc.gpsimd.dma_start(pl[:kw], pcol_d[r0:r0 + kw])
                xt = sb.tile([128, BT], dt.bfloat16, tag="xT")
                nc.sync.dma_start(xt[:kw], xT_d[r0:r0 + kw])
                a0t = sb.tile([128, NSH], dt.bfloat16, tag="a0")
                t_a = sb.tile([128, NSH], dt.float32, tag="tmp0c")
                t_b = sb.tile([128, NSH], dt.float32, tag="tmp0d")
                nc.vector.tensor_scalar(
                    t_a[:kw], pl[:kw, 0, :], wvec_t[:kw, 0:1], None, AL.mult)
                nc.vector.scalar_tensor_tensor(
                    t_b[:kw], pl[:kw, 1, :], wvec_t[:kw, 1:2], t_a[:kw],
                    AL.mult, AL.add)
                nc.vector.scalar_tensor_tensor(
                    a0t[:kw], pl[:kw, 2, :], wvec_t[:kw, 2:3], t_b[:kw],
                    AL.mult, AL.add)
                for m2 in range(2):
                    mw = 128 if m2 == 0 else NSH - 128
                    for c in range(CH):
                        nc.tensor.matmul(
                            ps1[m2 * CH + c][:mw, :CW],
                            a0t[:kw, m2 * 128:m2 * 128 + mw],
                            xt[:kw, c * CW:(c + 1) * CW],
                            start=(i == 0), stop=(i == NT - 1))

            # evacuate stage-1 psum -> bf16, ship to AllToAll bounce
            for m2 in range(2):
                mw = 128 if m2 == 0 else NSH - 128
                for c in range(CH):
                    nc.vector.tensor_copy(
                        y1sb[:mw, m2, c * CW:(c + 1) * CW],
                        ps1[m2 * CH + c][:mw, :CW])
            for c in range(NCORES):
                for m2 in range(2):
                    mw = 128 if m2 == 0 else NSH - 128
                    nc.sync.dma_start(
                        y1cc_in[c * NSH + m2 * 128:c * NSH + m2 * 128 + mw],
                        y1sb[:mw, m2, c * BTS:(c + 1) * BTS])
            if debug_taps:
                nc.sync.dma_start(dbg_y1[:], y1sb[:])
            nc.gpsimd.collective_compute(
                "AllToAll", AL.bypass, replica_groups=rg,
                ins=[y1cc_in[:]], outs=[y1cc_out[:]])
            nc.gpsimd.collective_compute(
                "AllGather", AL.bypass, replica_groups=rg,
                ins=[a2cc_in[:]], outs=[a2full[:]])

            # ---- stage 2: y2T_own[2000, 240] = b0^T @ y1T_own ----
            # 16 M-groups in 8 banks: a zero matmul (start=True) first
            # clears each bank and sets has_written on the full [128, 480]
            # region (WAW-ordered before the real matmuls); both real
            # M-groups then pure-accumulate with start=False.
            zt1 = res.tile([1, 128], dt.bfloat16, tag="zt1")
            nc.vector.memset(zt1[:], 0.0)
            zt2 = res.tile([1, 2 * BTS], dt.bfloat16, tag="zt2")
            nc.vector.memset(zt2[:], 0.0)
            ps2 = [ps.tile([128, 512], dt.float32, tag="psbig", name=f"ps2_{q}")
                   for q in range(8)]
            for q in range(8):
                nc.tensor.matmul(ps2[q][:, :2 * BTS], zt1[:], zt2[:],
                                 start=True, stop=False, skip_group_check=True)
            y2sb = res.tile([128, NT, BTS], dt.bfloat16, tag="y2sb")
            for i in range(NT):
                r0, kw = _ntile(i)
                brow = sb.tile([128, N], dt.bfloat16, tag="brow")
                for lo, hi in _row_segments(r0, kw):
                    nc.scalar.dma_start(brow[lo - r0:hi - r0, :],
                                        b0full[lo:hi, :])
                y1t = sb.tile([128, BTS], dt.bfloat16, tag="y1t")
                nc.sync.dma_start(y1t[:kw], y1cc_out[r0:r0 + kw])
                for mt in range(NT):
                    m0, mw = _ntile(mt)
                    nc.tensor.matmul(
                        ps2[mt // 2][:mw, (mt % 2) * BTS:(mt % 2) * BTS + BTS],
                        brow[:kw, m0:m0 + mw], y1t[:kw],
                        start=False, stop=(i == NT - 1),
                        skip_group_check=True)
            for mt in range(NT):
                m0, mw = _ntile(mt)
                nc.vector.tensor_copy(
                    y2sb[:mw, mt, :],
                    ps2[mt // 2][:mw, (mt % 2) * BTS:(mt % 2) * BTS + BTS])
            if debug_taps:
                nc.sync.dma_start(dbg_y2[:], y2sb[:])

            # ---- stage 3: y3T_own = a2^T @ y2T_own; fused enc ----
            ps3 = [ps.tile([128, 512], dt.float32, tag="psbig", name=f"ps3_{q}")
                   for q in range(8)]
            for q in range(8):
                nc.tensor.matmul(ps3[q][:, :2 * BTS], zt1[:], zt2[:],
                                 start=True, stop=False, skip_group_check=True)
            encT = res.tile([128, NT, BTS], dt.bfloat16, tag="encT")
            for i in range(NT):
                r0, kw = _ntile(i)
                arow = sb.tile([128, N], dt.bfloat16, tag="arow")
                for lo, hi in _row_segments(r0, kw):
                    nc.sync.dma_start(arow[lo - r0:hi - r0, :],
                                      a2full[lo:hi, :])
                for mt in range(NT):
                    m0, mw = _ntile(mt)
                    nc.tensor.matmul(
                        ps3[mt // 2][:mw, (mt % 2) * BTS:(mt % 2) * BTS + BTS],
                        arow[:kw, m0:m0 + mw], y2sb[:kw, i, :],
                        start=False, stop=(i == NT - 1),
                        skip_group_check=True)
            for mt in range(NT):
                m0, mw = _ntile(mt)
                xst = sb.tile([128, BTS], dt.bfloat16, tag="xst")
                nc.sync.dma_start(xst[:mw], xsT_d[m0:m0 + mw])
                t_e = sb.tile([128, BTS], dt.float32, tag="tmpenc")
                nc.vector.tensor_scalar(
                    t_e[:mw],
                    ps3[mt // 2][:mw, (mt % 2) * BTS:(mt % 2) * BTS + BTS],
                    thr_t[:mw, 0:1], 0.0, AL.subtract, AL.max)
                nc.vector.tensor_tensor(
                    encT[:mw, mt, :], t_e[:mw], xst[:mw], AL.add)
            if debug_taps:
                nc.sync.dma_start(dbg_enc[:], encT[:])

            # ---- basket^T[128, 240] = relu(lin_w @ enc^T + lin_b) ----
            pb = ps.tile([128, BTS], dt.float32, tag="psbig", name="pb")
            for i in range(NT):
                r0, kw = _ntile(i)
                lwt = sb.tile([128, D], dt.bfloat16, tag="lwt")
                nc.sync.dma_start(lwt[:kw], linwT_d[r0:r0 + kw])
                nc.tensor.matmul(pb[:], lwt[:kw], encT[:kw, i, :],
                                 start=(i == 0), stop=(i == NT - 1))
            bkT = res.tile([128, BTS], dt.bfloat16, tag="bkT")
            nc.scalar.activation(bkT[:], pb[:], AF.Relu,
                                 bias=linb_t[:, 0:1], scale=1.0)
            if debug_taps:
                nc.sync.dma_start(dbg_bk[:], bkT[:])

            # ---- gx[128, 4, 8, 30] = Wih @ basket^T + (bih+bhh) ----
            gx = res.tile([128, 4, S, BSH], dt.float32, tag="gx")
            for g in range(4):
                pg = ps.tile([128, S, BSH], dt.float32, tag="psbig",
                             name=f"pg_{g}")
                nc.tensor.matmul(pg[:], wih_t[:, g * 128:(g + 1) * 128], bkT[:],
                                 start=True, stop=True)
                nc.vector.tensor_scalar(
                    gx[:, g], pg[:], biasp_t[:, g:g + 1], None, AL.add)

            # ---- LSTM over S steps, gates order [i, f, o, g] ----
            # gx layout [128, 4, BSH, S]? No: basketT free dim is (t, b')
            # t-major, so pg free is (t, b2) = [S, BSH]; gx[:, g] holds
            # (t, b') blocks; step t slices gx[:, g, t, :].
            ys = res.tile([128, S, BSH], dt.float16, tag="ys")
            cT = res.tile([128, BSH], dt.float32, tag="cT")
            nc.sync.dma_start(cT[:], c0t_d[:])
            h0f = res.tile([128, BSH], dt.float16, tag="h0f")
            h0tmp = sb.tile([128, BSH], dt.float32, tag="h0tmp")
            nc.sync.dma_start(h0tmp[:], h0t_d[:])
            nc.vector.tensor_copy(h0f[:], h0tmp[:])
            for t in range(S):
                hprev = h0f[:] if t == 0 else ys[:, t - 1, :]
                pghs = [ps.tile([128, BSH], dt.float32, tag="psbig",
                                name=f"pgh_{t}_{g}") for g in range(4)]
                for g in range(4):
                    nc.tensor.matmul(pghs[g][:], whh_t[:, g * 128:(g + 1) * 128],
                                     hprev, start=True, stop=True)
                gall = sb.tile([128, 4, BSH], dt.float32, tag="gall")
                for g in range(4):
                    nc.vector.tensor_tensor(gall[:, g], pghs[g][:],
                                            gx[:, g, t, :], AL.add)
                sg = sb.tile([128, 3, BSH], dt.float32, tag="sg")
                nc.scalar.activation(sg[:], gall[:, 0:3], AF.Sigmoid)
                tg = sb.tile([128, BSH], dt.float32, tag="tg")
                nc.scalar.activation(tg[:], gall[:, 3], AF.Tanh)
                t1 = sb.tile([128, BSH], dt.float32, tag="t1")
                nc.vector.tensor_tensor(t1[:], sg[:, 1], cT[:], AL.mult)
                t2 = sb.tile([128, BSH], dt.float32, tag="t2")
                nc.vector.tensor_tensor(t2[:], sg[:, 0], tg[:], AL.mult)
                nc.vector.tensor_tensor(cT[:], t1[:], t2[:], AL.add)
                tc_ = sb.tile([128, BSH], dt.float32, tag="tc_")
                nc.scalar.activation(tc_[:], cT[:], AF.Tanh)
                nc.vector.tensor_tensor(ys[:, t, :], sg[:, 2], tc_[:], AL.mult)

            # last h per batch: mask-select then add-tree over t
            # (exactly one t is selected per (u, b'), so grouping is free)
            lmul = res.tile([128, S, BSH], dt.float16, tag="lmul")
            nc.vector.tensor_tensor(lmul[:], ys[:], mask_t[:], AL.mult)
            r10 = res.tile([128, 10, BSH], dt.float16, tag="r10")
            nc.vector.tensor_tensor(r10[:], lmul[:, 0:10], lmul[:, 10:20], AL.add)
            nc.vector.tensor_tensor(r10[:], r10[:], lmul[:, 20:30], AL.add)
            r5 = res.tile([128, 5, BSH], dt.float16, tag="r5")
            nc.vector.tensor_tensor(r5[:], r10[:, 0:5], r10[:, 5:10], AL.add)
            r2 = res.tile([128, 2, BSH], dt.float16, tag="r2")
            nc.vector.tensor_tensor(r2[:], r5[:, 0:2], r5[:, 2:4], AL.add)
            lastf = res.tile([128, 1, BSH], dt.float16, tag="lastf")
            nc.vector.tensor_tensor(lastf[:], r2[:, 0:1], r2[:, 1:2], AL.add)
            nc.vector.tensor_tensor(lastf[:], lastf[:], r5[:, 4:5], AL.add)
            if debug_taps:
                nc.sync.dma_start(dbg_last[:], lastf[:, 0, :])

            # ---- scoring: predict[8, 2000] ----
            prb = res.tile([BSH, N], dt.float32, tag="prb")
            for c in range(4):
                wst = sb.tile([128, N // 4], dt.float16, tag="wst")
                nc.sync.dma_start(wst[:], wsc_d[:, c * (N // 4):(c + 1) * (N // 4)])
                psc = ps.tile([BSH, N // 4], dt.float32, tag="psbig",
                              name=f"psc_{c}")
                nc.tensor.matmul(psc[:], lastf[:, 0, :], wst[:],
                                 start=True, stop=True)
                nc.scalar.activation(prb[:, c * (N // 4):(c + 1) * (N // 4)],
                                     psc[:], AF.Sigmoid)
            outsb = res.tile([BSH, N], dt.float32, tag="outsb")
            nc.vector.tensor_tensor(outsb[:], prb[:], factor_t[:], AL.mult)
            nc.sync.dma_start(out_d[:], outsb[:])

    nc.compile()
    return nc


def _softmax_row0(w):
    w = np.asarray(w, np.float64)
    m = w - w.max(axis=1, keepdims=True)
    e = np.exp(m)
    p = e / e.sum(axis=1, keepdims=True)
    return p[0].astype(np.float32)


def _host_prep(inputs):
    return _host_prep_args(**inputs)


def _host_prep_args(A, seq_len, seqs, h0, c0, W1a, W1b, W2, lin_w, lin_b,
                    Wih, Whh, bih, bhh, Wscore, I_B, threshold):
    A = np.asarray(A, np.float32)
    seqs = np.asarray(seqs, np.float32)
    seq_len = np.asarray(seq_len).astype(np.int64)
    sa = _softmax_row0(W1a)
    sb_ = _softmax_row0(W1b)
    s2 = _softmax_row0(W2)
    scale = np.maximum(np.asarray(I_B, np.float32), 0.0)

    x = seqs.reshape(BT, N)
    # column order: block c (core) outer, then (t, b') t-major within block
    cperm = np.empty(BT, np.int64)
    for c_ in range(NCORES):
        for t_ in range(S):
            for b2 in range(BSH):
                cperm[c_ * BTS + t_ * BSH + b2] = (c_ * BSH + b2) * S + t_
    xTp = x.T[:, cperm]
    xT = np.ascontiguousarray(xTp).astype(_bf16)
    xsT = np.ascontiguousarray(xTp * scale[:, None]).astype(_bf16)
    wvec = np.tile(np.concatenate([sa, sb_, s2])[None, :], (128, 1)).astype(np.float32)
    thr = np.full((128, 1), np.asarray(threshold, np.float32).ravel()[0], np.float32)
    linwT = np.ascontiguousarray(np.asarray(lin_w, np.float32).T).astype(_bf16)
    linb = np.asarray(lin_b, np.float32).reshape(128, 1)

    perm = np.r_[0:128, 128:256, 384:512, 256:384]  # [i, f, o, g]
    wih = np.ascontiguousarray(np.asarray(Wih, np.float32)[perm].T).astype(_bf16)
    whh = np.ascontiguousarray(np.asarray(Whh, np.float32)[perm].T).astype(np.float16)
    biasp = np.ascontiguousarray(
        (np.asarray(bih, np.float32) + np.asarray(bhh, np.float32))[perm]
        .reshape(4, 128).T).astype(np.float32)
    wsc = np.ascontiguousarray(np.asarray(Wscore, np.float32).T).astype(np.float16)
    factor = np.tile((0.5 + 0.5 * scale)[None, :], (BSH, 1)).astype(np.float32)

    in_maps = []
    for k in range(NCORES):
        ck = slice(k * NSH, (k + 1) * NSH)
        bk = slice(k * BSH, (k + 1) * BSH)
        pcol = np.ascontiguousarray(A[:, ck, :].transpose(0, 2, 1)).astype(_bf16)
        prow = np.ascontiguousarray(A[ck, :, :].transpose(0, 2, 1)).astype(_bf16)
        mm = np.zeros((S, BSH), np.float16)
        sl = seq_len[bk]
        for b2 in range(BSH):
            mm[sl[b2] - 1, b2] = 1.0
        in_maps.append({
            "pcol": pcol,
            "prow": prow,
            "xT": xT,
            "xsT": np.ascontiguousarray(xsT[:, k * BTS:(k + 1) * BTS]),
            "wvec": wvec,
            "thr": thr,
            "linwT": linwT,
            "linb": linb,
            "wih": wih,
            "whh": whh,
            "biasp": biasp,
            "wsc": wsc,
            "factor": factor,
            "h0t": np.ascontiguousarray(np.asarray(h0, np.float32)[0, bk].T),
            "c0t": np.ascontiguousarray(np.asarray(c0, np.float32)[0, bk].T),
            "maskt": np.tile(mm.reshape(1, S, BSH), (128, 1, 1)).astype(np.float16),
        })
    return in_maps


def kernel(A, seq_len, seqs, h0, c0, W1a, W1b, W2, lin_w, lin_b,
           Wih, Whh, bih, bhh, Wscore, I_B, threshold):
    from concourse.bass_utils import run_bass_kernel_spmd

    in_maps = _host_prep_args(A, seq_len, seqs, h0, c0, W1a, W1b, W2,
                              lin_w, lin_b, Wih, Whh, bih, bhh, Wscore,
                              I_B, threshold)
    nc = _build_program()
    res = run_bass_kernel_spmd(nc, in_maps, list(range(NCORES)))
    out = np.concatenate([res.results[k]["predict"] for k in range(NCORES)], axis=0)
    return out.astype(np.float32)


# revision 16
# speedup vs baseline: 214.6347x; 1.0374x over previous
"""GTN-Rec fused kernel for 8 Trainium2 NeuronCores.

Math (only channel 0 of H is consumed downstream):
    a0 = A @ softmax(W1a)[0]; b0 = A @ softmax(W1b)[0]; a2 = A @ softmax(W2)[0]
    y3 = ((x @ a0) @ b0) @ a2          (x = seqs flattened [B*S, N])
    enc = x*relu(I_B) + relu(y3 - thr)
    basket = relu(enc @ lin_w.T + lin_b) -> LSTM -> take h[seq_len-1]
    predict = sigmoid(last @ Wscore.T) * (0.5 + 0.5*relu(I_B))

Sharding: a0 column-sharded (N/8=250 cols per core) for stage 1
(y1T_k = a0_k^T x^T over all B*S); one AllToAll re-shards the stage-1
output from column- to batch-sharded (240 of B*S per core); b0/a2 are
produced ROW-sharded and AllGathered early so stages 2-3 run the full
contraction locally with no further activation collectives; LSTM +
scoring run batch-sharded (8 of 64 batches per core); the host
concatenates the row shards.

Chain matmuls in bf16 w/ fp32 PSUM (end-to-end rel err ~4e-4 vs fp32
reference); LSTM pocket in fp16.
"""
import numpy as np
import ml_dtypes

N, E, C, D, U, B, S = 2000, 3, 2, 128, 128, 64, 30
BT = B * S            # 1920
NCORES = 8
NSH = N // NCORES     # 250
BSH = B // NCORES     # 8
BTS = BSH * S         # 240
NT = (N + 127) // 128  # 16 K/M tiles over N (15x128 + 80)
CH = 4                # moving chunks for 1920-wide stage-1 output
CW = BT // CH         # 480

_bf16 = ml_dtypes.bfloat16


def _ntile(i):
    """(row0, width) of the i-th 128-row tile over N=2000."""
    r0 = i * 128
    return r0, min(128, N - r0)


def _row_segments(r0, kw):
    """Split rows [r0, r0+kw) at NSH-sized core-block boundaries."""
    segs = []
    r = r0
    while r < r0 + kw:
        hi = min(r0 + kw, (r // NSH + 1) * NSH)
        segs.append((r, hi))
        r = hi
    return segs


def _build_program(debug_taps=False):
    import concourse.mybir as mybir
    import concourse.tile as tile
    from concourse import bacc

    dt = mybir.dt
    AL = mybir.AluOpType
    AF = mybir.ActivationFunctionType
    nc = bacc.Bacc("TRN2", target_bir_lowering=False, debug=False,
                   num_devices=NCORES)

    # ---- I/O ----
    pcol_d = nc.dram_tensor("pcol", [N, E, NSH], dt.bfloat16, kind="ExternalInput")
    prow_d = nc.dram_tensor("prow", [NSH, E, N], dt.bfloat16, kind="ExternalInput")
    xT_d = nc.dram_tensor("xT", [N, BT], dt.bfloat16, kind="ExternalInput")
    xsT_d = nc.dram_tensor("xsT", [N, BTS], dt.bfloat16, kind="ExternalInput")
    wvec_d = nc.dram_tensor("wvec", [128, 3 * E], dt.float32, kind="ExternalInput")
    thr_d = nc.dram_tensor("thr", [128, 1], dt.float32, kind="ExternalInput")
    linwT_d = nc.dram_tensor("linwT", [N, D], dt.bfloat16, kind="ExternalInput")
    linb_d = nc.dram_tensor("linb", [128, 1], dt.float32, kind="ExternalInput")
    wih_d = nc.dram_tensor("wih", [128, 4 * U], dt.bfloat16, kind="ExternalInput")
    whh_d = nc.dram_tensor("whh", [128, 4 * U], dt.float16, kind="ExternalInput")
    biasp_d = nc.dram_tensor("biasp", [128, 4], dt.float32, kind="ExternalInput")
    wsc_d = nc.dram_tensor("wsc", [128, N], dt.float16, kind="ExternalInput")
    factor_d = nc.dram_tensor("factor", [BSH, N], dt.float32, kind="ExternalInput")
    h0t_d = nc.dram_tensor("h0t", [128, BSH], dt.float32, kind="ExternalInput")
    c0t_d = nc.dram_tensor("c0t", [128, BSH], dt.float32, kind="ExternalInput")
    mask_d = nc.dram_tensor("maskt", [128, S, BSH], dt.float16, kind="ExternalInput")
    out_d = nc.dram_tensor("predict", [BSH, N], dt.float32, kind="ExternalOutput")
    if debug_taps:
        dbg_y1 = nc.dram_tensor("dbg_y1", [128, 2, BT], dt.bfloat16, kind="ExternalOutput")
        dbg_y2 = nc.dram_tensor("dbg_y2", [128, NT, BTS], dt.bfloat16, kind="ExternalOutput")
        dbg_enc = nc.dram_tensor("dbg_enc", [128, NT, BTS], dt.bfloat16, kind="ExternalOutput")
        dbg_bk = nc.dram_tensor("dbg_bk", [128, BTS], dt.bfloat16, kind="ExternalOutput")
        dbg_last = nc.dram_tensor("dbg_last", [128, BSH], dt.float16, kind="ExternalOutput")

    # ---- internal DRAM (collective bounce) ----
    b0cc_in = nc.dram_tensor("b0cc_in", [NSH, N], dt.bfloat16)
    a2cc_in = nc.dram_tensor("a2cc_in", [NSH + 1, N], dt.bfloat16)
    b0full = nc.dram_tensor("b0full", [N, N], dt.bfloat16, addr_space="Shared")
    a2full = nc.dram_tensor("a2full", [NCORES * (NSH + 1), N], dt.bfloat16,
                            addr_space="Shared")
    y1cc_in = nc.dram_tensor("y1cc_in", [N, BTS], dt.bfloat16)
    y1cc_out = nc.dram_tensor("y1cc_out", [N, BTS], dt.bfloat16)

    rg = [list(range(NCORES))]

    with tile.TileContext(nc) as tc:
        with tc.tile_pool(name="sb", bufs=3) as sb, \
             tc.tile_pool(name="res", bufs=1) as res, \
             tc.tile_pool(name="ps", bufs=8, space="PSUM") as ps:

            # ---- constants in SBUF ----
            wvec_t = res.tile([128, 3 * E], dt.float32, tag="wvec")
            nc.sync.dma_start(wvec_t[:], wvec_d[:])
            thr_t = res.tile([128, 1], dt.float32, tag="thr")
            nc.sync.dma_start(thr_t[:], thr_d[:])
            linb_t = res.tile([128, 1], dt.float32, tag="linb")
            nc.sync.dma_start(linb_t[:], linb_d[:])
            wih_t = res.tile([128, 4 * U], dt.bfloat16, tag="wih")
            nc.sync.dma_start(wih_t[:], wih_d[:])
            whh_t = res.tile([128, 4 * U], dt.float16, tag="whh")
            nc.sync.dma_start(whh_t[:], whh_d[:])
            biasp_t = res.tile([128, 4], dt.float32, tag="biasp")
            nc.sync.dma_start(biasp_t[:], biasp_d[:])
            mask_t = res.tile([128, S, BSH], dt.float16, tag="mask")
            nc.sync.dma_start(mask_t[:], mask_d[:])
            factor_t = res.tile([BSH, N], dt.float32, tag="factor")
            nc.sync.dma_start(factor_t[:], factor_d[:])

            # ---- stage 0 (rows): b0/a2 row shards -> early AllGathers ----
            for rt in range(2):
                p0, pw = (0, 128) if rt == 0 else (128, NSH - 128)
                for half in range(2):
                    hs = slice(half * 1000, half * 1000 + 1000)
                    plr = sb.tile([128, E, 1000], dt.bfloat16, tag="prow", bufs=2)
                    nc.scalar.dma_start(plr[:pw], prow_d[p0:p0 + pw, :, hs])
                    for j, cc in ((1, b0cc_in), (2, a2cc_in)):
                        ot = sb.tile([128, 1000], dt.bfloat16, tag="rowout", bufs=2)
                        t_a = sb.tile([128, 1000], dt.float32, tag="tmp0a", bufs=2)
                        t_b = sb.tile([128, 1000], dt.float32, tag="tmp0b", bufs=2)
                        nc.vector.tensor_scalar(
                            t_a[:pw], plr[:pw, 0, :],
                            wvec_t[:pw, 3 * j:3 * j + 1], None, AL.mult)
                        nc.vector.scalar_tensor_tensor(
                            t_b[:pw], plr[:pw, 1, :],
                            wvec_t[:pw, 3 * j + 1:3 * j + 2], t_a[:pw],
                            AL.mult, AL.add)
                        nc.vector.scalar_tensor_tensor(
                            ot[:pw], plr[:pw, 2, :],
                            wvec_t[:pw, 3 * j + 2:3 * j + 3], t_b[:pw],
                            AL.mult, AL.add)
                        nc.scalar.dma_start(cc[p0:p0 + pw, hs], ot[:pw])

            nc.gpsimd.collective_compute(
                "AllGather", AL.bypass, replica_groups=rg,
                ins=[b0cc_in[:]], outs=[b0full[:]])

            # ---- stage 0 (cols): a0 shard; stage 1: y1T_k = a0_k^T @ x^T ----
            # xT and pcol resident (loaded up front) so stage 1 runs with
            # no mid-stage DMA dependencies.
            xtres = res.tile([128, NT, BT], dt.bfloat16, tag="xtres")
            a0res = res.tile([128, NT, NSH], dt.bfloat16, tag="a0res")
            for i in range(NT):
                r0, kw = _ntile(i)
                nc.sync.dma_start(xtres[:kw, i], xT_d[r0:r0 + kw])
                pl = sb.tile([128, E, NSH], dt.bfloat16, tag="pcol")
                nc.gpsimd.dma_start(pl[:kw], pcol_d[r0:r0 + kw])
                t_a = sb.tile([128, NSH], dt.float32, tag="tmp0c")
                t_b = sb.tile([128, NSH], dt.float32, tag="tmp0d")
                nc.vector.tensor_scalar(
                    t_a[:kw], pl[:kw, 0, :], wvec_t[:kw, 0:1], None, AL.mult)
                nc.vector.scalar_tensor_tensor(
                    t_b[:kw], pl[:kw, 1, :], wvec_t[:kw, 1:2], t_a[:kw],
                    AL.mult, AL.add)
                nc.vector.scalar_tensor_tensor(
                    a0res[:kw, i], pl[:kw, 2, :], wvec_t[:kw, 2:3], t_b[:kw],
                    AL.mult, AL.add)
            y1sb = res.tile([128, 2, BT], dt.bfloat16, tag="y1sb")
            ps1 = [ps.tile([128, 512], dt.float32, tag="psbig", name=f"ps1_{q}")
                   for q in range(8)]
            for i in range(NT):
                r0, kw = _ntile(i)
                for m2 in range(2):
                    mw = 128 if m2 == 0 else NSH - 128
                    for c in range(CH):
                        nc.tensor.matmul(
                            ps1[m2 * CH + c][:mw, :CW],
                            a0res[:kw, i, m2 * 128:m2 * 128 + mw],
                            xtres[:kw, i, c * CW:(c + 1) * CW],
                            start=(i == 0), stop=(i == NT - 1))

            # evacuate stage-1 psum -> bf16, ship to AllToAll bounce
            for m2 in range(2):
                mw = 128 if m2 == 0 else NSH - 128
                for c in range(CH):
                    nc.vector.tensor_copy(
                        y1sb[:mw, m2, c * CW:(c + 1) * CW],
                        ps1[m2 * CH + c][:mw, :CW])
            for c in range(NCORES):
                for m2 in range(2):
                    mw = 128 if m2 == 0 else NSH - 128
                    nc.sync.dma_start(
                        y1cc_in[c * NSH + m2 * 128:c * NSH + m2 * 128 + mw],
                        y1sb[:mw, m2, c * BTS:(c + 1) * BTS])
            if debug_taps:
                nc.sync.dma_start(dbg_y1[:], y1sb[:])
            nc.gpsimd.collective_compute(
                "AllToAll", AL.bypass, replica_groups=rg,
                ins=[y1cc_in[:]], outs=[y1cc_out[:]])
            # pad row of a2cc_in copied from the A2A output: a real data
            # dependency that pins the a2 AllGather after the AllToAll in
            # the collective stream.
            nc.sync.dma_start(a2cc_in[NSH:NSH + 1, 0:BTS], y1cc_out[0:1, :])
            nc.gpsimd.collective_compute(
                "AllGather", AL.bypass, replica_groups=rg,
                ins=[a2cc_in[:]], outs=[a2full[:]])

            # ---- stage 2: y2T_own[2000, 240] = b0^T @ y1T_own ----
            # 16 M-groups in 8 banks: a zero matmul (start=True) first
            # clears each bank and sets has_written on the full [128, 480]
            # region (WAW-ordered before the real matmuls); both real
            # M-groups then pure-accumulate with start=False.
            zt1 = res.tile([1, 128], dt.bfloat16, tag="zt1")
            nc.vector.memset(zt1[:], 0.0)
            zt2 = res.tile([1, 2 * BTS], dt.bfloat16, tag="zt2")
            nc.vector.memset(zt2[:], 0.0)
            ps2 = [ps.tile([128, 512], dt.float32, tag="psbig", name=f"ps2_{q}")
                   for q in range(8)]
            for q in range(8):
                nc.tensor.matmul(ps2[q][:, :2 * BTS], zt1[:], zt2[:],
                                 start=True, stop=False, skip_group_check=True)
            y2sb = res.tile([128, NT, BTS], dt.bfloat16, tag="y2sb")
            for i in range(NT):
                r0, kw = _ntile(i)
                brow = sb.tile([128, N], dt.bfloat16, tag="brow")
                for lo, hi in _row_segments(r0, kw):
                    nc.scalar.dma_start(brow[lo - r0:hi - r0, :],
                                        b0full[lo:hi, :])
                y1t = sb.tile([128, BTS], dt.bfloat16, tag="y1t")
                nc.sync.dma_start(y1t[:kw], y1cc_out[r0:r0 + kw])
                for mt in range(NT):
                    m0, mw = _ntile(mt)
                    nc.tensor.matmul(
                        ps2[mt // 2][:mw, (mt % 2) * BTS:(mt % 2) * BTS + BTS],
                        brow[:kw, m0:m0 + mw], y1t[:kw],
                        start=False, stop=(i == NT - 1),
                        skip_group_check=True)
            for mt in range(NT):
                m0, mw = _ntile(mt)
                nc.vector.tensor_copy(
                    y2sb[:mw, mt, :],
                    ps2[mt // 2][:mw, (mt % 2) * BTS:(mt % 2) * BTS + BTS])
            if debug_taps:
                nc.sync.dma_start(dbg_y2[:], y2sb[:])

            # ---- stage 3: y3T_own = a2^T @ y2T_own; fused enc ----
            ps3 = [ps.tile([128, 512], dt.float32, tag="psbig", name=f"ps3_{q}")
                   for q in range(8)]
            for q in range(8):
                nc.tensor.matmul(ps3[q][:, :2 * BTS], zt1[:], zt2[:],
                                 start=True, stop=False, skip_group_check=True)
            encT = res.tile([128, NT, BTS], dt.bfloat16, tag="encT")
            for i in range(NT):
                r0, kw = _ntile(i)
                arow = sb.tile([128, N], dt.bfloat16, tag="arow")
                for lo, hi in _row_segments(r0, kw):
                    c = lo // NSH
                    nc.sync.dma_start(
                        arow[lo - r0:hi - r0, :],
                        a2full[c * (NSH + 1) + lo - c * NSH:
                               c * (NSH + 1) + hi - c * NSH, :])
                for mt in range(NT):
                    m0, mw = _ntile(mt)
                    nc.tensor.matmul(
                        ps3[mt // 2][:mw, (mt % 2) * BTS:(mt % 2) * BTS + BTS],
                        arow[:kw, m0:m0 + mw], y2sb[:kw, i, :],
                        start=False, stop=(i == NT - 1),
                        skip_group_check=True)
            for mt in range(NT):
                m0, mw = _ntile(mt)
                xst = sb.tile([128, BTS], dt.bfloat16, tag="xst")
                nc.sync.dma_start(xst[:mw], xsT_d[m0:m0 + mw])
                t_e = sb.tile([128, BTS], dt.float32, tag="tmpenc")
                nc.vector.tensor_scalar(
                    t_e[:mw],
                    ps3[mt // 2][:mw, (mt % 2) * BTS:(mt % 2) * BTS + BTS],
                    thr_t[:mw, 0:1], 0.0, AL.subtract, AL.max)
                nc.vector.tensor_tensor(
                    encT[:mw, mt, :], t_e[:mw], xst[:mw], AL.add)
            if debug_taps:
                nc.sync.dma_start(dbg_enc[:], encT[:])

            # ---- basket^T[128, 240] = relu(lin_w @ enc^T + lin_b) ----
            pb = ps.tile([128, BTS], dt.float32, tag="psbig", name="pb")
            for i in range(NT):
                r0, kw = _ntile(i)
                lwt = sb.tile([128, D], dt.bfloat16, tag="lwt")
                nc.sync.dma_start(lwt[:kw], linwT_d[r0:r0 + kw])
                nc.tensor.matmul(pb[:], lwt[:kw], encT[:kw, i, :],
                                 start=(i == 0), stop=(i == NT - 1))
            bkT = res.tile([128, BTS], dt.bfloat16, tag="bkT")
            nc.scalar.activation(bkT[:], pb[:], AF.Relu,
                                 bias=linb_t[:, 0:1], scale=1.0)
            if debug_taps:
                nc.sync.dma_start(dbg_bk[:], bkT[:])

            # ---- gx[128, 4, 8, 30] = Wih @ basket^T + (bih+bhh) ----
            gx = res.tile([128, 4, S, BSH], dt.float32, tag="gx")
            for g in range(4):
                pg = ps.tile([128, S, BSH], dt.float32, tag="psbig",
                             name=f"pg_{g}")
                nc.tensor.matmul(pg[:], wih_t[:, g * 128:(g + 1) * 128], bkT[:],
                                 start=True, stop=True)
                nc.vector.tensor_scalar(
                    gx[:, g], pg[:], biasp_t[:, g:g + 1], None, AL.add)

            # ---- LSTM over S steps, gates order [i, f, o, g] ----
            # gx layout [128, 4, BSH, S]? No: basketT free dim is (t, b')
            # t-major, so pg free is (t, b2) = [S, BSH]; gx[:, g] holds
            # (t, b') blocks; step t slices gx[:, g, t, :].
            ys = res.tile([128, S, BSH], dt.float16, tag="ys")
            cT = res.tile([128, BSH], dt.float32, tag="cT")
            nc.sync.dma_start(cT[:], c0t_d[:])
            h0f = res.tile([128, BSH], dt.float16, tag="h0f")
            h0tmp = sb.tile([128, BSH], dt.float32, tag="h0tmp")
            nc.sync.dma_start(h0tmp[:], h0t_d[:])
            nc.vector.tensor_copy(h0f[:], h0tmp[:])
            # cT holds 2*c ("cd"); tanh(v) = 2*sigmoid(2v) - 1 everywhere,
            # with the g-gate rows of Wih/Whh/bias pre-doubled on the host.
            for t in range(S):
                hprev = h0f[:] if t == 0 else ys[:, t - 1, :]
                pgh = ps.tile([128, 4, BSH], dt.float32, tag="psbig",
                              name=f"pgh_{t}")
                nc.tensor.matmul(pgh[:, :, :], zt1[:], zt2[:, :4 * BSH],
                                 start=True, stop=False, skip_group_check=True)
                for g in range(4):
                    nc.tensor.matmul(pgh[:, g], whh_t[:, g * 128:(g + 1) * 128],
                                     hprev, start=False, stop=True,
                                     skip_group_check=True)
                gall = sb.tile([128, 4, BSH], dt.float32, tag="gall")
                nc.vector.tensor_tensor(gall[:], pgh[:], gx[:, :, t, :], AL.add)
                sg = sb.tile([128, 4, BSH], dt.float32, tag="sg")
                nc.scalar.activation(sg[:], gall[:], AF.Sigmoid)
                tg2 = sb.tile([128, BSH], dt.float32, tag="tg2")
                nc.vector.tensor_scalar(tg2[:], sg[:, 3], 4.0, -2.0,
                                        AL.mult, AL.add)
                t1 = sb.tile([128, BSH], dt.float32, tag="t1")
                nc.vector.tensor_tensor(t1[:], sg[:, 1], cT[:], AL.mult)
                t2 = sb.tile([128, BSH], dt.float32, tag="t2")
                nc.vector.tensor_tensor(t2[:], sg[:, 0], tg2[:], AL.mult)
                nc.vector.tensor_tensor(cT[:], t1[:], t2[:], AL.add)
                sc_ = sb.tile([128, BSH], dt.float32, tag="sc_")
                nc.scalar.activation(sc_[:], cT[:], AF.Sigmoid)
                u_ = sb.tile([128, BSH], dt.float32, tag="u_")
                nc.vector.tensor_scalar(u_[:], sc_[:], 2.0, -1.0,
                                        AL.mult, AL.add)
                nc.vector.tensor_tensor(ys[:, t, :], sg[:, 2], u_[:], AL.mult)

            # last h per batch: mask-select then add-tree over t
            # (exactly one t is selected per (u, b'), so grouping is free)
            lmul = res.tile([128, S, BSH], dt.float16, tag="lmul")
            nc.vector.tensor_tensor(lmul[:], ys[:], mask_t[:], AL.mult)
            r10 = res.tile([128, 10, BSH], dt.float16, tag="r10")
            nc.vector.tensor_tensor(r10[:], lmul[:, 0:10], lmul[:, 10:20], AL.add)
            nc.vector.tensor_tensor(r10[:], r10[:], lmul[:, 20:30], AL.add)
            r5 = res.tile([128, 5, BSH], dt.float16, tag="r5")
            nc.vector.tensor_tensor(r5[:], r10[:, 0:5], r10[:, 5:10], AL.add)
            r2 = res.tile([128, 2, BSH], dt.float16, tag="r2")
            nc.vector.tensor_tensor(r2[:], r5[:, 0:2], r5[:, 2:4], AL.add)
            lastf = res.tile([128, 1, BSH], dt.float16, tag="lastf")
            nc.vector.tensor_tensor(lastf[:], r2[:, 0:1], r2[:, 1:2], AL.add)
            nc.vector.tensor_tensor(lastf[:], lastf[:], r5[:, 4:5], AL.add)
            if debug_taps:
                nc.sync.dma_start(dbg_last[:], lastf[:, 0, :])

            # ---- scoring: predict[8, 2000] ----
            prb = res.tile([BSH, N], dt.float32, tag="prb")
            for c in range(4):
                wst = sb.tile([128, N // 4], dt.float16, tag="wst")
                nc.sync.dma_start(wst[:], wsc_d[:, c * (N // 4):(c + 1) * (N // 4)])
                psc = ps.tile([BSH, N // 4], dt.float32, tag="psbig",
                              name=f"psc_{c}")
                nc.tensor.matmul(psc[:], lastf[:, 0, :], wst[:],
                                 start=True, stop=True)
                nc.scalar.activation(prb[:, c * (N // 4):(c + 1) * (N // 4)],
                                     psc[:], AF.Sigmoid)
            nc.vector.tensor_tensor(prb[:], prb[:], factor_t[:], AL.mult)
            nc.sync.dma_start(out_d[:], prb[:])

    nc.compile()
    return nc


def _softmax_row0(w):
    w = np.asarray(w, np.float64)
    m = w - w.max(axis=1, keepdims=True)
    e = np.exp(m)
    p = e / e.sum(axis=1, keepdims=True)
    return p[0].astype(np.float32)


def _host_prep(inputs):
    return _host_prep_args(**inputs)


def _host_prep_args(A, seq_len, seqs, h0, c0, W1a, W1b, W2, lin_w, lin_b,
                    Wih, Whh, bih, bhh, Wscore, I_B, threshold):
    A = np.asarray(A, np.float32)
    seqs = np.asarray(seqs, np.float32)
    seq_len = np.asarray(seq_len).astype(np.int64)
    sa = _softmax_row0(W1a)
    sb_ = _softmax_row0(W1b)
    s2 = _softmax_row0(W2)
    scale = np.maximum(np.asarray(I_B, np.float32), 0.0)

    x = seqs.reshape(BT, N)
    # column order: block c (core) outer, then (t, b') t-major within block
    cperm = np.empty(BT, np.int64)
    for c_ in range(NCORES):
        for t_ in range(S):
            for b2 in range(BSH):
                cperm[c_ * BTS + t_ * BSH + b2] = (c_ * BSH + b2) * S + t_
    xTp = x.T[:, cperm]
    xT = np.ascontiguousarray(xTp).astype(_bf16)
    xsT = np.ascontiguousarray(xTp * scale[:, None]).astype(_bf16)
    wvec = np.tile(np.concatenate([sa, sb_, s2])[None, :], (128, 1)).astype(np.float32)
    thr = np.full((128, 1), np.asarray(threshold, np.float32).ravel()[0], np.float32)
    linwT = np.ascontiguousarray(np.asarray(lin_w, np.float32).T).astype(_bf16)
    linb = np.asarray(lin_b, np.float32).reshape(128, 1)

    perm = np.r_[0:128, 128:256, 384:512, 256:384]  # [i, f, o, g]
    # g-gate rows doubled: tanh(g) is computed as 2*sigmoid(2g) - 1
    gd = np.ones((512, 1), np.float32)
    gd[384:512] = 2.0
    wih = np.ascontiguousarray((np.asarray(Wih, np.float32)[perm] * gd).T).astype(_bf16)
    whh = np.ascontiguousarray((np.asarray(Whh, np.float32)[perm] * gd).T).astype(np.float16)
    biasp = np.ascontiguousarray(
        ((np.asarray(bih, np.float32) + np.asarray(bhh, np.float32))[perm]
         * gd[:, 0]).reshape(4, 128).T).astype(np.float32)
    wsc = np.ascontiguousarray(np.asarray(Wscore, np.float32).T).astype(np.float16)
    factor = np.tile((0.5 + 0.5 * scale)[None, :], (BSH, 1)).astype(np.float32)

    in_maps = []
    for k in range(NCORES):
        ck = slice(k * NSH, (k + 1) * NSH)
        bk = slice(k * BSH, (k + 1) * BSH)
        pcol = np.ascontiguousarray(A[:, ck, :].transpose(0, 2, 1)).astype(_bf16)
        prow = np.ascontiguousarray(A[ck, :, :].transpose(0, 2, 1)).astype(_bf16)
        mm = np.zeros((S, BSH), np.float16)
        sl = seq_len[bk]
        for b2 in range(BSH):
            mm[sl[b2] - 1, b2] = 1.0
        in_maps.append({
            "pcol": pcol,
            "prow": prow,
            "xT": xT,
            "xsT": np.ascontiguousarray(xsT[:, k * BTS:(k + 1) * BTS]),
            "wvec": wvec,
            "thr": thr,
            "linwT": linwT,
            "linb": linb,
            "wih": wih,
            "whh": whh,
            "biasp": biasp,
            "wsc": wsc,
            "factor": factor,
            "h0t": np.ascontiguousarray(np.asarray(h0, np.float32)[0, bk].T),
            "c0t": np.ascontiguousarray(2.0 * np.asarray(c0, np.float32)[0, bk].T),
            "maskt": np.tile(mm.reshape(1, S, BSH), (128, 1, 1)).astype(np.float16),
        })
    return in_maps


def kernel(A, seq_len, seqs, h0, c0, W1a, W1b, W2, lin_w, lin_b,
           Wih, Whh, bih, bhh, Wscore, I_B, threshold):
    from concourse.bass_utils import run_bass_kernel_spmd

    in_maps = _host_prep_args(A, seq_len, seqs, h0, c0, W1a, W1b, W2,
                              lin_w, lin_b, Wih, Whh, bih, bhh, Wscore,
                              I_B, threshold)
    nc = _build_program()
    res = run_bass_kernel_spmd(nc, in_maps, list(range(NCORES)))
    out = np.concatenate([res.results[k]["predict"] for k in range(NCORES)], axis=0)
    return out.astype(np.float32)


# revision 20
# speedup vs baseline: 242.8557x; 1.1315x over previous
"""GTN-Rec fused kernel for 8 Trainium2 NeuronCores.

Math (only channel 0 of H is consumed downstream):
    a0 = A @ softmax(W1a)[0]; b0 = A @ softmax(W1b)[0]; a2 = A @ softmax(W2)[0]
    y3 = ((x @ a0) @ b0) @ a2          (x = seqs flattened [B*S, N])
    enc = x*relu(I_B) + relu(y3 - thr)
    basket = relu(enc @ lin_w.T + lin_b) -> LSTM -> take h[seq_len-1]
    predict = sigmoid(last @ Wscore.T) * (0.5 + 0.5*relu(I_B))

Sharding: a0 column-sharded (250 cols/core) for stage 1; one AllToAll
re-shards the stage-1 output from column- to batch-sharded (240 of B*S
per core); b0/a2 are produced ROW-sharded and AllGathered early so
stages 2-3 run full contractions locally; LSTM + scoring batch-sharded
(8 of 64 batches per core); host concatenates the row shards.

b0's AllGather is split at the M-tile boundary (1024/976 columns) to
stay under the ~1MB mesh-algorithm crossover; a2's AllGather carries a
pad row copied from the AllToAll output, a real data dependency that
pins it after the AllToAll in the single collective stream.

Chain matmuls bf16 + fp32 PSUM (end-to-end ~1e-4 vs fp32 ref);
LSTM pocket fp16.
"""
import numpy as np
import ml_dtypes

N, E, C, D, U, B, S = 2000, 3, 2, 128, 128, 64, 30
BT = B * S            # 1920
NCORES = 8
NSH = N // NCORES     # 250
BSH = B // NCORES     # 8
BTS = BSH * S         # 240
NT = (N + 127) // 128  # 16 tiles over N (15x128 + 80)
CH = 4
CW = BT // CH         # 480
NA = 1024             # b0 AllGather column split (M-tile aligned)
NB = N - NA           # 976

_bf16 = ml_dtypes.bfloat16


def _ntile(i):
    r0 = i * 128
    return r0, min(128, N - r0)


def _arow_segments(r0, kw):
    """a2full rows with one pad row per NSH block: (dst_lo, dst_hi, src_row)."""
    segs = []
    r = r0
    while r < r0 + kw:
        c = r // NSH
        hi = min(r0 + kw, (c + 1) * NSH)
        segs.append((r - r0, hi - r0, c * (NSH + 1) + (r - c * NSH)))
        r = hi
    return segs


def _tile128(a, inner_shape):
    """[R, ...] -> [128, ceil(R/128), ...] zero-padded, partition-major."""
    R = a.shape[0]
    nt = (R + 127) // 128
    out = np.zeros((nt * 128,) + a.shape[1:], a.dtype)
    out[:R] = a
    out = out.reshape((nt, 128) + a.shape[1:])
    return np.ascontiguousarray(np.moveaxis(out, 0, 1))


def _build_program(debug_taps=False):
    import concourse.mybir as mybir
    import concourse.tile as tile
    from concourse import bacc

    dt = mybir.dt
    AL = mybir.AluOpType
    AF = mybir.ActivationFunctionType
    nc = bacc.Bacc("TRN2", target_bir_lowering=False, debug=False,
                   num_devices=NCORES)

    # ---- I/O (host-tiled layouts; one DMA each) ----
    pc4_d = nc.dram_tensor("pc4", [128, NT, E, NSH], dt.bfloat16, kind="ExternalInput")
    pr4_d = nc.dram_tensor("pr4", [128, 2, E, N], dt.bfloat16, kind="ExternalInput")
    xt4_d = nc.dram_tensor("xt4", [128, NT, BT], dt.bfloat16, kind="ExternalInput")
    xs4_d = nc.dram_tensor("xs4", [128, NT, BTS], dt.bfloat16, kind="ExternalInput")
    lw4_d = nc.dram_tensor("lw4", [128, NT, D], dt.bfloat16, kind="ExternalInput")
    cst_d = nc.dram_tensor("cst", [128, 15], dt.float32, kind="ExternalInput")
    wih_d = nc.dram_tensor("wih", [128, 4 * U], dt.bfloat16, kind="ExternalInput")
    whh_d = nc.dram_tensor("whh", [128, 4 * U], dt.float16, kind="ExternalInput")
    wsc_d = nc.dram_tensor("wsc", [128, N], dt.float16, kind="ExternalInput")
    factor_d = nc.dram_tensor("factor", [BSH, N], dt.float16, kind="ExternalInput")
    h0t_d = nc.dram_tensor("h0t", [128, BSH], dt.float32, kind="ExternalInput")
    c0t_d = nc.dram_tensor("c0t", [128, BSH], dt.float32, kind="ExternalInput")
    mask_d = nc.dram_tensor("maskt", [128, S, BSH], dt.float16, kind="ExternalInput")
    out_d = nc.dram_tensor("predict", [BSH, N], dt.float32, kind="ExternalOutput")
    if debug_taps:
        dbg_y1 = nc.dram_tensor("dbg_y1", [128, 2, BT], dt.bfloat16, kind="ExternalOutput")
        dbg_y2 = nc.dram_tensor("dbg_y2", [128, NT, BTS], dt.bfloat16, kind="ExternalOutput")
        dbg_enc = nc.dram_tensor("dbg_enc", [128, NT, BTS], dt.bfloat16, kind="ExternalOutput")
        dbg_bk = nc.dram_tensor("dbg_bk", [128, BTS], dt.bfloat16, kind="ExternalOutput")
        dbg_last = nc.dram_tensor("dbg_last", [128, BSH], dt.float16, kind="ExternalOutput")

    # ---- internal DRAM (collective bounce) ----
    b0ccA = nc.dram_tensor("b0ccA", [NSH, NA], dt.bfloat16)
    b0ccB = nc.dram_tensor("b0ccB", [NSH, NB], dt.bfloat16)
    b0fullA = nc.dram_tensor("b0fullA", [N, NA], dt.bfloat16, addr_space="Shared")
    b0fullB = nc.dram_tensor("b0fullB", [N, NB], dt.bfloat16, addr_space="Shared")
    a2cc_in = nc.dram_tensor("a2cc_in", [NSH + 1, N], dt.bfloat16)
    a2full = nc.dram_tensor("a2full", [NCORES * (NSH + 1), N], dt.bfloat16,
                            addr_space="Shared")
    y1cc_in = nc.dram_tensor("y1cc_in", [N, BTS], dt.bfloat16)
    y1cc_out = nc.dram_tensor("y1cc_out", [N, BTS], dt.bfloat16)

    rg = [list(range(NCORES))]

    with tile.TileContext(nc) as tc:
        with tc.tile_pool(name="sb", bufs=3) as sb, \
             tc.tile_pool(name="res", bufs=1) as res, \
             tc.tile_pool(name="ps", bufs=8, space="PSUM") as ps:

            # ---- resident loads (one DMA each) ----
            cst_t = res.tile([128, 15], dt.float32, tag="cst")
            nc.sync.dma_start(cst_t[:], cst_d[:])
            wv = cst_t  # wvec cols 0:9, thr 9, linb 10, biasp 11:15
            wih_t = res.tile([128, 4 * U], dt.bfloat16, tag="wih")
            nc.sync.dma_start(wih_t[:], wih_d[:])
            whh_t = res.tile([128, 4 * U], dt.float16, tag="whh")
            nc.sync.dma_start(whh_t[:], whh_d[:])
            mask_t = res.tile([128, S, BSH], dt.float16, tag="mask")
            nc.sync.dma_start(mask_t[:], mask_d[:])
            factor_t = sb.tile([BSH, N], dt.float16, tag="factor", bufs=1)
            nc.sync.dma_start(factor_t[:], factor_d[:])
            wsc_t = res.tile([128, N], dt.float16, tag="wsc")
            nc.sync.dma_start(wsc_t[:], wsc_d[:])
            xs4_t = res.tile([128, NT, BTS], dt.bfloat16, tag="xs4")
            nc.sync.dma_start(xs4_t[:], xs4_d[:])
            lw4_t = res.tile([128, NT, D], dt.bfloat16, tag="lw4")
            nc.sync.dma_start(lw4_t[:], lw4_d[:])
            xtres = res.tile([128, NT, BT], dt.bfloat16, tag="xtres")
            nc.sync.dma_start(xtres[:], xt4_d[:])

            # ---- stage 0 (rows): b0/a2 row shards -> early AllGathers ----
            for rt in range(2):
                p0, pw = (0, 128) if rt == 0 else (128, NSH - 128)
                for half, h0_, hw in ((0, 0, NA), (1, NA, NB)):
                    plr = sb.tile([128, E, NA], dt.bfloat16, tag="prow", bufs=2)
                    nc.scalar.dma_start(plr[:pw, :, :hw],
                                        pr4_d[:pw, rt, :, h0_:h0_ + hw])
                    for j, ccs in ((1, (b0ccA, b0ccB)), (2, None)):
                        ot = sb.tile([128, NA], dt.bfloat16, tag="rowout", bufs=2)
                        t_a = sb.tile([128, NA], dt.float32, tag="tmp0a", bufs=2)
                        t_b = sb.tile([128, NA], dt.float32, tag="tmp0b", bufs=2)
                        nc.vector.tensor_scalar(
                            t_a[:pw, :hw], plr[:pw, 0, :hw],
                            wv[:pw, 3 * j:3 * j + 1], None, AL.mult)
                        nc.vector.scalar_tensor_tensor(
                            t_b[:pw, :hw], plr[:pw, 1, :hw],
                            wv[:pw, 3 * j + 1:3 * j + 2], t_a[:pw, :hw],
                            AL.mult, AL.add)
                        nc.vector.scalar_tensor_tensor(
                            ot[:pw, :hw], plr[:pw, 2, :hw],
                            wv[:pw, 3 * j + 2:3 * j + 3], t_b[:pw, :hw],
                            AL.mult, AL.add)
                        if j == 1:
                            cc = ccs[half]
                            nc.scalar.dma_start(cc[p0:p0 + pw, :], ot[:pw, :hw])
                        else:
                            nc.scalar.dma_start(
                                a2cc_in[p0:p0 + pw, h0_:h0_ + hw], ot[:pw, :hw])

            nc.gpsimd.collective_compute(
                "AllGather", AL.bypass, replica_groups=rg,
                ins=[b0ccA[:]], outs=[b0fullA[:]])
            nc.gpsimd.collective_compute(
                "AllGather", AL.bypass, replica_groups=rg,
                ins=[b0ccB[:]], outs=[b0fullB[:]])

            # ---- stage 0 (cols): a0; stage 1: y1T_k = a0_k^T @ x^T ----
            a0res = res.tile([128, NT, NSH], dt.bfloat16, tag="a0res")
            for i in range(NT):
                r0, kw = _ntile(i)
                plc = sb.tile([128, E, NSH], dt.bfloat16, tag="plc")
                nc.gpsimd.dma_start(plc[:kw], pc4_d[:, i][:kw])
                t_a = sb.tile([128, NSH], dt.float32, tag="tmp0c")
                t_b = sb.tile([128, NSH], dt.float32, tag="tmp0d")
                nc.vector.tensor_scalar(
                    t_a[:kw], plc[:kw, 0, :], wv[:kw, 0:1], None, AL.mult)
                nc.vector.scalar_tensor_tensor(
                    t_b[:kw], plc[:kw, 1, :], wv[:kw, 1:2], t_a[:kw],
                    AL.mult, AL.add)
                nc.vector.scalar_tensor_tensor(
                    a0res[:kw, i], plc[:kw, 2, :], wv[:kw, 2:3], t_b[:kw],
                    AL.mult, AL.add)
            y1sb = res.tile([128, 2, NCORES, BTS], dt.bfloat16, tag="y1sb")
            ps1 = [ps.tile([128, 512], dt.float32, tag="psbig", name=f"ps1_{q}")
                   for q in range(8)]
            for i in range(NT):
                r0, kw = _ntile(i)
                for m2 in range(2):
                    mw = 128 if m2 == 0 else NSH - 128
                    for c in range(CH):
                        nc.tensor.matmul(
                            ps1[m2 * CH + c][:mw, :CW],
                            a0res[:kw, i, m2 * 128:m2 * 128 + mw],
                            xtres[:kw, i, c * CW:(c + 1) * CW],
                            start=(i == 0), stop=(i == NT - 1))

            # evacuate stage-1 psum -> bf16, ship to AllToAll bounce
            for m2 in range(2):
                mw = 128 if m2 == 0 else NSH - 128
                for c8 in range(NCORES):
                    nc.vector.tensor_copy(
                        y1sb[:mw, m2, c8, :],
                        ps1[m2 * CH + c8 // 2][:mw,
                                               (c8 % 2) * BTS:(c8 % 2) * BTS + BTS])
            # y1cc_in rows c*NSH + m2*128 + p <- y1sb[p, m2, c, :]: one
            # strided DMA per m2 (dst viewed [r, c, j])
            y1v = y1cc_in.ap().rearrange("(c r) j -> r c j", c=NCORES)
            for m2 in range(2):
                mw = 128 if m2 == 0 else NSH - 128
                nc.sync.dma_start(y1v[m2 * 128:m2 * 128 + mw], y1sb[:mw, m2])
            if debug_taps:
                nc.sync.dma_start(dbg_y1[:], y1sb[:])
            nc.gpsimd.collective_compute(
                "AllToAll", AL.bypass, replica_groups=rg,
                ins=[y1cc_in[:]], outs=[y1cc_out[:]])
            # pad row of a2cc_in from the A2A output: pins a2's AllGather
            # after the AllToAll in the collective stream
            nc.sync.dma_start(a2cc_in[NSH:NSH + 1, 0:BTS], y1cc_out[0:1, :])
            nc.gpsimd.collective_compute(
                "AllGather", AL.bypass, replica_groups=rg,
                ins=[a2cc_in[:]], outs=[a2full[:]])

            # ---- stage 2: y2T_own[2000, 240] = b0^T @ y1T_own ----
            zt1 = res.tile([1, 128], dt.bfloat16, tag="zt1")
            nc.vector.memset(zt1[:], 0.0)
            zt2 = res.tile([1, 2 * BTS], dt.bfloat16, tag="zt2")
            nc.vector.memset(zt2[:], 0.0)
            ps2 = [ps.tile([128, 512], dt.float32, tag="psbig", name=f"ps2_{q}")
                   for q in range(8)]
            for q in range(8):
                nc.tensor.matmul(ps2[q][:, :2 * BTS], zt1[:], zt2[:],
                                 start=True, stop=False, skip_group_check=True)
            y2sb = res.tile([128, NT, BTS], dt.bfloat16, tag="y2sb")
            for i in range(NT):
                r0, kw = _ntile(i)
                brow = sb.tile([128, N], dt.bfloat16, tag="brow")
                nc.scalar.dma_start(brow[:kw, :NA], b0fullA[r0:r0 + kw, :])
                nc.scalar.dma_start(brow[:kw, NA:], b0fullB[r0:r0 + kw, :])
                y1t = sb.tile([128, BTS], dt.bfloat16, tag="y1t")
                nc.sync.dma_start(y1t[:kw], y1cc_out[r0:r0 + kw])
                for mt in range(NT):
                    m0, mw = _ntile(mt)
                    nc.tensor.matmul(
                        ps2[mt // 2][:mw, (mt % 2) * BTS:(mt % 2) * BTS + BTS],
                        brow[:kw, m0:m0 + mw], y1t[:kw],
                        start=False, stop=(i == NT - 1),
                        skip_group_check=True)
            for mt in range(NT):
                m0, mw = _ntile(mt)
                nc.vector.tensor_copy(
                    y2sb[:mw, mt, :],
                    ps2[mt // 2][:mw, (mt % 2) * BTS:(mt % 2) * BTS + BTS])
            if debug_taps:
                nc.sync.dma_start(dbg_y2[:], y2sb[:])

            # ---- stage 3: y3T_own = a2^T @ y2T_own; fused enc ----
            ps3 = [ps.tile([128, 512], dt.float32, tag="psbig", name=f"ps3_{q}")
                   for q in range(8)]
            for q in range(8):
                nc.tensor.matmul(ps3[q][:, :2 * BTS], zt1[:], zt2[:],
                                 start=True, stop=False, skip_group_check=True)
            encT = res.tile([128, NT, BTS], dt.bfloat16, tag="encT")
            for i in range(NT):
                r0, kw = _ntile(i)
                arow = sb.tile([128, N], dt.bfloat16, tag="arow")
                for dlo, dhi, srow in _arow_segments(r0, kw):
                    nc.sync.dma_start(arow[dlo:dhi, :],
                                      a2full[srow:srow + dhi - dlo, :])
                for mt in range(NT):
                    m0, mw = _ntile(mt)
                    nc.tensor.matmul(
                        ps3[mt // 2][:mw, (mt % 2) * BTS:(mt % 2) * BTS + BTS],
                        arow[:kw, m0:m0 + mw], y2sb[:kw, i, :],
                        start=False, stop=(i == NT - 1),
                        skip_group_check=True)
            for mt in range(NT):
                m0, mw = _ntile(mt)
                t_e = sb.tile([128, BTS], dt.float32, tag="tmpenc")
                nc.vector.tensor_scalar(
                    t_e[:mw],
                    ps3[mt // 2][:mw, (mt % 2) * BTS:(mt % 2) * BTS + BTS],
                    wv[:mw, 9:10], 0.0, AL.subtract, AL.max)
                nc.vector.tensor_tensor(
                    encT[:mw, mt, :], t_e[:mw], xs4_t[:mw, mt, :], AL.add)
            if debug_taps:
                nc.sync.dma_start(dbg_enc[:], encT[:])

            # ---- basket^T[128, 240] = relu(lin_w @ enc^T + lin_b) ----
            pb = ps.tile([128, BTS], dt.float32, tag="psbig", name="pb")
            for i in range(NT):
                r0, kw = _ntile(i)
                nc.tensor.matmul(pb[:], lw4_t[:kw, i, :], encT[:kw, i, :],
                                 start=(i == 0), stop=(i == NT - 1))
            bkT = res.tile([128, BTS], dt.bfloat16, tag="bkT")
            nc.scalar.activation(bkT[:], pb[:], AF.Relu,
                                 bias=wv[:, 10:11], scale=1.0)
            if debug_taps:
                nc.sync.dma_start(dbg_bk[:], bkT[:])

            # ---- gx[128, 4, S, BSH] = Wih @ basket^T + (bih+bhh) ----
            gx = res.tile([128, 4, S, BSH], dt.float32, tag="gx")
            for g in range(4):
                pg = ps.tile([128, S, BSH], dt.float32, tag="psbig",
                             name=f"pg_{g}")
                nc.tensor.matmul(pg[:], wih_t[:, g * 128:(g + 1) * 128], bkT[:],
                                 start=True, stop=True)
                nc.vector.tensor_scalar(
                    gx[:, g], pg[:], wv[:, 11 + g:12 + g], None, AL.add)

            # ---- LSTM, gates [i, f, o, g]; cT holds 2*c; tanh via ACT scale
            # gx_t is DVE-copied into the psum bank over a zero matmul
            # (has_written set), the 4 gate matmuls accumulate on top, and
            # sigma reads PSUM directly -> one less hop per step.
            ys = res.tile([128, S, BSH], dt.float16, tag="ys")
            cT = res.tile([128, BSH], dt.float32, tag="cT")
            nc.sync.dma_start(cT[:], c0t_d[:])
            h0f = res.tile([128, BSH], dt.float16, tag="h0f")
            h0tmp = sb.tile([128, BSH], dt.float32, tag="h0tmp")
            nc.sync.dma_start(h0tmp[:], h0t_d[:])
            nc.vector.tensor_copy(h0f[:], h0tmp[:])
            for t in range(S):
                hprev = h0f[:] if t == 0 else ys[:, t - 1, :]
                pgh = ps.tile([128, 4, BSH], dt.float32, tag="psbig",
                              name=f"pgh_{t}")
                nc.tensor.matmul(pgh[:, :, :], zt1[:], zt2[:, :4 * BSH],
                                 start=True, stop=False, skip_group_check=True)
                nc.vector.tensor_copy(pgh[:], gx[:, :, t, :])
                for g in range(4):
                    nc.tensor.matmul(pgh[:, g], whh_t[:, g * 128:(g + 1) * 128],
                                     hprev, start=False, stop=True,
                                     skip_group_check=True)
                sg = sb.tile([128, 4, BSH], dt.float32, tag="sg")
                nc.scalar.activation(sg[:], pgh[:], AF.Sigmoid)
                tg2 = sb.tile([128, BSH], dt.float32, tag="tg2")
                nc.vector.tensor_scalar(tg2[:], sg[:, 3], 4.0, -2.0,
                                        AL.mult, AL.add)
                t1 = sb.tile([128, BSH], dt.float32, tag="t1")
                nc.vector.tensor_tensor(t1[:], sg[:, 1], cT[:], AL.mult)
                t2 = sb.tile([128, BSH], dt.float32, tag="t2")
                nc.vector.tensor_tensor(t2[:], sg[:, 0], tg2[:], AL.mult)
                nc.vector.tensor_tensor(cT[:], t1[:], t2[:], AL.add)
                tc_ = sb.tile([128, BSH], dt.float32, tag="tc_")
                nc.scalar.activation(tc_[:], cT[:], AF.Tanh, scale=0.5)
                nc.vector.tensor_tensor(ys[:, t, :], sg[:, 2], tc_[:], AL.mult)

            # last h per batch: mask-select then add-tree over t
            lmul = res.tile([128, S, BSH], dt.float16, tag="lmul")
            nc.vector.tensor_tensor(lmul[:], ys[:], mask_t[:], AL.mult)
            r10 = res.tile([128, 10, BSH], dt.float16, tag="r10")
            nc.vector.tensor_tensor(r10[:], lmul[:, 0:10], lmul[:, 10:20], AL.add)
            nc.vector.tensor_tensor(r10[:], r10[:], lmul[:, 20:30], AL.add)
            r5 = res.tile([128, 5, BSH], dt.float16, tag="r5")
            nc.vector.tensor_tensor(r5[:], r10[:, 0:5], r10[:, 5:10], AL.add)
            r2 = res.tile([128, 2, BSH], dt.float16, tag="r2")
            nc.vector.tensor_tensor(r2[:], r5[:, 0:2], r5[:, 2:4], AL.add)
            lastf = res.tile([128, 1, BSH], dt.float16, tag="lastf")
            nc.vector.tensor_tensor(lastf[:], r2[:, 0:1], r2[:, 1:2], AL.add)
            nc.vector.tensor_tensor(lastf[:], lastf[:], r5[:, 4:5], AL.add)
            if debug_taps:
                nc.sync.dma_start(dbg_last[:], lastf[:, 0, :])

            # ---- scoring: predict[8, 2000] ----
            prb = sb.tile([BSH, N], dt.float32, tag="prb", bufs=1)
            for c in range(4):
                psc = ps.tile([BSH, N // 4], dt.float32, tag="psbig",
                              name=f"psc_{c}")
                nc.tensor.matmul(psc[:], lastf[:, 0, :],
                                 wsc_t[:, c * (N // 4):(c + 1) * (N // 4)],
                                 start=True, stop=True)
                nc.scalar.activation(prb[:, c * (N // 4):(c + 1) * (N // 4)],
                                     psc[:], AF.Sigmoid)
            nc.vector.tensor_tensor(prb[:], prb[:], factor_t[:], AL.mult)
            nc.sync.dma_start(out_d[:], prb[:])

    nc.compile()
    return nc


def _softmax_row0(w):
    w = np.asarray(w, np.float64)
    m = w - w.max(axis=1, keepdims=True)
    e = np.exp(m)
    p = e / e.sum(axis=1, keepdims=True)
    return p[0].astype(np.float32)


def _host_prep(inputs):
    return _host_prep_args(**inputs)


def _host_prep_args(A, seq_len, seqs, h0, c0, W1a, W1b, W2, lin_w, lin_b,
                    Wih, Whh, bih, bhh, Wscore, I_B, threshold):
    A = np.asarray(A, np.float32)
    seqs = np.asarray(seqs, np.float32)
    seq_len = np.asarray(seq_len).astype(np.int64)
    sa = _softmax_row0(W1a)
    sb_ = _softmax_row0(W1b)
    s2 = _softmax_row0(W2)
    scale = np.maximum(np.asarray(I_B, np.float32), 0.0)

    x = seqs.reshape(BT, N)
    # column order: block c (core) outer, then (t, b') t-major within block
    cperm = np.empty(BT, np.int64)
    for c_ in range(NCORES):
        for t_ in range(S):
            for b2 in range(BSH):
                cperm[c_ * BTS + t_ * BSH + b2] = (c_ * BSH + b2) * S + t_
    xTp = np.ascontiguousarray(x.T[:, cperm])
    xt4 = _tile128(xTp.astype(_bf16), None)          # [128, NT, BT]
    xsTp = xTp * scale[:, None]
    linwT = np.ascontiguousarray(np.asarray(lin_w, np.float32).T)
    lw4 = _tile128(linwT.astype(_bf16), None)        # [128, NT, D]

    cst = np.zeros((128, 15), np.float32)
    cst[:, 0:9] = np.concatenate([sa, sb_, s2])[None, :]
    cst[:, 9] = np.asarray(threshold, np.float32).ravel()[0]
    cst[:, 10] = np.asarray(lin_b, np.float32)
    perm = np.r_[0:128, 128:256, 384:512, 256:384]   # [i, f, o, g]
    gd = np.ones((512, 1), np.float32)
    gd[384:512] = 2.0                                # tanh(g) = 2*sig(2g)-1
    cst[:, 11:15] = ((np.asarray(bih, np.float32)
                      + np.asarray(bhh, np.float32))[perm]
                     * gd[:, 0]).reshape(4, 128).T
    wih = np.ascontiguousarray((np.asarray(Wih, np.float32)[perm] * gd).T).astype(_bf16)
    whh = np.ascontiguousarray((np.asarray(Whh, np.float32)[perm] * gd).T).astype(np.float16)
    wsc = np.ascontiguousarray(np.asarray(Wscore, np.float32).T).astype(np.float16)
    factor = np.tile((0.5 + 0.5 * scale)[None, :], (BSH, 1)).astype(np.float16)

    in_maps = []
    for k in range(NCORES):
        ck = slice(k * NSH, (k + 1) * NSH)
        bk = slice(k * BSH, (k + 1) * BSH)
        pcol = np.ascontiguousarray(A[:, ck, :].transpose(0, 2, 1)).astype(_bf16)
        prow = np.ascontiguousarray(A[ck, :, :].transpose(0, 2, 1)).astype(_bf16)
        mm = np.zeros((S, BSH), np.float16)
        sl = seq_len[bk]
        for b2 in range(BSH):
            mm[sl[b2] - 1, b2] = 1.0
        in_maps.append({
            "pc4": _tile128(pcol, None),             # [128, NT, E, NSH]
            "pr4": _tile128(prow, None),             # [128, 2, E, N]
            "xt4": xt4,
            "xs4": _tile128(
                np.ascontiguousarray(xsTp[:, k * BTS:(k + 1) * BTS]).astype(_bf16),
                None),
            "lw4": lw4,
            "cst": cst,
            "wih": wih,
            "whh": whh,
            "wsc": wsc,
            "factor": factor,
            "h0t": np.ascontiguousarray(np.asarray(h0, np.float32)[0, bk].T),
            "c0t": np.ascontiguousarray(2.0 * np.asarray(c0, np.float32)[0, bk].T),
            "maskt": np.tile(mm.reshape(1, S, BSH), (128, 1, 1)).astype(np.float16),
        })
    return in_maps


def kernel(A, seq_len, seqs, h0, c0, W1a, W1b, W2, lin_w, lin_b,
           Wih, Whh, bih, bhh, Wscore, I_B, threshold):
    from concourse.bass_utils import run_bass_kernel_spmd

    in_maps = _host_prep_args(A, seq_len, seqs, h0, c0, W1a, W1b, W2,
                              lin_w, lin_b, Wih, Whh, bih, bhh, Wscore,
                              I_B, threshold)
    nc = _build_program()
    res = run_bass_kernel_spmd(nc, in_maps, list(range(NCORES)))
    out = np.concatenate([res.results[k]["predict"] for k in range(NCORES)], axis=0)
    return out.astype(np.float32)
